# revision 30
# baseline (speedup 1.0000x reference)
"""Trainium2 Bass kernel for FConv2d (FFT conv module), v5.

out = irfftn( rfftn(x, axes=(c,h,w)) * rfftn(pad(weight)) )[:, :, ::4] reshaped.

Device math (data-parallel over batch, 4 per core x 8 cores), bf16 data with
fp32 PSUM accumulation:
  W   on-device weight transform: spatial DFT of the 9x9 taps (contract 81)
      then channel DFT (contract 16), Gauss-packed into wpk for the MUL stage.
  T   on-device u8->bf16 shift + transpose of x[b] ([c, hw] -> [hw, c]
      chunks) via PE identity matmuls; host ships x in its native layout.
  S   joint 2D spatial rFFT while data is REAL: contract hw=1024 via 8
      PSUM-accumulated matmuls per batch; the per-(b, c) x-dequant scale is
      folded into the PSUM evacuation.
  C   channel DFT (contract c=128) producing X[k_c, f] for 544 spatial freqs.
  MUL complex multiply with W-hat via Gauss 3-mult on DVE (+Pool offload).
  I1  subsampled inverse channel DFT via paired matmuls.
  I2  joint 2D spatial inverse + Re extraction via paired matmuls, then
      per-channel abs-max u8 quantization of the output on device.

The wall-clock bottleneck is host<->device traffic: the NeuronCores sit
behind an axon tunnel moving ~35-50 MB/s with ~80 ms per-call dispatch
latency, while the device compute itself is ~0.2 ms. Per call this kernel
ships one 4.5 MB H2D payload (u8-quantized x + bf16 weight bytes + f32
scales) and one 8.9 MB D2H result (u8-quantized out + f32 per-channel
scales, dequantized on host). All DFT twiddle constants and the output
zero buffers are weight-independent, device-resident, and uploaded only
once; the jitted shard_map executable is cached across calls.
"""
import sys
import numpy as np

for _p in ("/opt/trn_rl_repo", "/root/.axon_site/_ro/trn_rl_repo"):
    if _p not in sys.path:
        sys.path.insert(0, _p)

import ml_dtypes

import concourse.bacc as bacc
import concourse.bass as bass
import concourse.mybir as mybir
import concourse.tile as tile
from concourse import bass2jax

F32 = mybir.dt.float32
BF16 = mybir.dt.bfloat16
U8 = mybir.dt.uint8
NPBF = ml_dtypes.bfloat16

# u8 output quantization: u = out * (QCLIP/absmax) + QOFF on device (DVE
# float->u8 convert), out = (u - QOFF) / inv on host. QCLIP < 127 leaves
# headroom so saturation/rounding at the extremes cannot wrap.
QCLIP = 126.5
QOFF = 128.0

B = 32          # full batch
B_LOC = 4       # per core
N_CORES = 8
CIN = 128
L = 32
NFIL = 8        # num filters n
NF = 544        # stored spatial freqs (32 * 17)
NFP = 640       # padded: 5 chunks of 128
XROWS = B_LOC * CIN          # 512 rows of u8-quantized x per core
WROWS = 32                   # 32 rows holding the packed [128,128] bf16 weight
SROWS = 2                    # 2 rows holding the 512 f32 dequant scales
PROWS = XROWS + WROWS + SROWS   # 546 payload rows per core

# x u8 quantization: q = round(x * 127/absmax) + 128 per (b, c) row on host;
# device subtracts 128 during the u8->bf16 convert and folds absmax/127 into
# the spatial-rFFT output (per-partition scale).
XQ = 127.0

# which n-pairs get their zr/zi add/sub on Pool instead of DVE
POOL_PAIRS = (0, 1, 2, 3)


# ----------------------------------------------------------------- constants
def build_consts():
    """Weight-independent DFT factor tensors (bf16). Uploaded once."""
    f = np.arange(NF)
    p = f // 17
    q = f % 17

    # S: joint real 2D rfft factor, per hw chunk t: [128 hw, 1088]
    # cols = [Re(f) 0:544 | Im(f) 544:1088];  Re = cos, Im = -sin
    f2 = np.zeros((128, 8, 1088), dtype=np.float64)
    hw_p = np.arange(128)
    for t in range(8):
        h = 4 * t + hw_p // 32
        w = hw_p % 32
        ang = 2 * np.pi * (np.outer(h, p) + np.outer(w, q)) / 32.0
        f2[:, t, 0:NF] = np.cos(ang)
        f2[:, t, NF:2 * NF] = -np.sin(ang)

    # C: channel DFT lhsT pack [c, 384] = [cos | sin | -sin]
    c = np.arange(128)
    k = np.arange(128)
    angc = 2 * np.pi * np.outer(c, k) / 128.0
    fc = np.concatenate(
        [np.cos(angc), np.sin(angc), -np.sin(angc)], axis=1)

    # I1 rhs: e1 = [cos|sin], e2 = [-sin|cos] at output positions c=4j
    j32 = np.arange(32)
    ange = 2 * np.pi * np.outer(k, j32) / 32.0
    er = np.cos(ange)
    ei = np.sin(ange)
    e1 = np.concatenate([er, ei], axis=1)
    e2 = np.concatenate([-ei, er], axis=1)

    # I2 rhs: k2d[128, 5, 2, 1024]: cos / -sin of inverse angles
    k2d = np.zeros((128, 5, 2, 1024), dtype=np.float64)
    yz = np.arange(1024)
    y = yz // 32
    z = yz % 32
    fp = np.arange(NFP)
    valid = (fp < NF).astype(np.float64)
    pv = np.zeros(NFP, dtype=np.int64)
    qv = np.zeros(NFP, dtype=np.int64)
    pv[:NF] = p
    qv[:NF] = q
    for fcb in range(5):
        sl = slice(fcb * 128, (fcb + 1) * 128)
        ang4 = 2 * np.pi * (np.outer(pv[sl], y) + np.outer(qv[sl], z)) / 32.0
        k2d[:, fcb, 0, :] = np.cos(ang4) * valid[sl][:, None]
        k2d[:, fcb, 1, :] = -np.sin(ang4) * valid[sl][:, None]

    # W stage step 1: spatial DFT of the 9x9 taps; rows = k1*9+k2 (pad to
    # 128), cols = [Re 544 | Im 544], with alpha[q]/(128*32*32) folded in.
    alpha = np.full(17, 2.0)
    alpha[0] = 1.0
    alpha[16] = 1.0
    scale = alpha[q] / (128.0 * 32.0 * 32.0)
    k12 = np.arange(81)
    k1 = k12 // 9
    k2 = k12 % 9
    angw = 2 * np.pi * (np.outer(k1, p) + np.outer(k2, q)) / 32.0
    fw = np.zeros((128, 1088), dtype=np.float64)
    fw[:81, 0:NF] = np.cos(angw) * scale
    fw[:81, NF:2 * NF] = -np.sin(angw) * scale

    # W stage step 2: channel DFT lhsT pack over cin'=16: [16, 384] =
    # [cos | sin | -sin] of 2*pi*c*cin'/128
    ci = np.arange(16)
    angec = 2 * np.pi * np.outer(ci, c) / 128.0
    ec = np.concatenate(
        [np.cos(angec), np.sin(angec), -np.sin(angec)], axis=1)

    # T stage: identity for PE transposes
    id128 = np.eye(128)

    cast = lambda a: np.ascontiguousarray(a).astype(NPBF)
    return {
        "f2": cast(f2),
        "fc": cast(fc),
        "e1": cast(e1),
        "e2": cast(e2),
        "k2d": cast(k2d),
        "fw": cast(fw),
        "ec": cast(ec),
        "id128": cast(id128),
    }


# ----------------------------------------------------------------- program
def build_program():
    nc = bacc.Bacc("TRN2", target_bir_lowering=False, debug=False)
    # xw rows 0:512 = u8-quantized x[b] as [c, h*w] (native layout); rows
    # 512:544 = weight packed [128, 128] bf16 (wT[k1*9+k2, n*16+cin'],
    # zero-padded) as raw bytes; rows 544:546 = the 512 f32 x-dequant scales
    # laid out [c, b].
    xw_d = nc.dram_tensor("xw", [PROWS, 1024], U8, kind="ExternalInput")
    f2_d = nc.dram_tensor("f2", [128, 8, 1088], BF16, kind="ExternalInput")
    fc_d = nc.dram_tensor("fc", [128, 384], BF16, kind="ExternalInput")
    e1_d = nc.dram_tensor("e1", [128, 64], BF16, kind="ExternalInput")
    e2_d = nc.dram_tensor("e2", [128, 64], BF16, kind="ExternalInput")
    k2d_d = nc.dram_tensor("k2d", [128, 5, 2, 1024], BF16, kind="ExternalInput")
    fw_d = nc.dram_tensor("fw", [128, 1088], BF16, kind="ExternalInput")
    ec_d = nc.dram_tensor("ec", [16, 384], BF16, kind="ExternalInput")
    id_d = nc.dram_tensor("id128", [128, 128], BF16, kind="ExternalInput")
    # rows 0:256 = u8-quantized out channels; row 256 = the 256 per-channel
    # f32 inverse scales (bitcast into the u8 row).
    out_d = nc.dram_tensor("out", [B_LOC, 257, 1024], U8,
                           kind="ExternalOutput")

    with tile.TileContext(nc) as tc:
        with (
            tc.tile_pool(name="consts", bufs=1) as cpool,
            tc.tile_pool(name="xin", bufs=2) as xpool,
            tc.tile_pool(name="xs", bufs=2) as xspool,
            tc.tile_pool(name="xc", bufs=2) as xcpool,
            tc.tile_pool(name="z", bufs=2) as zpool,
            tc.tile_pool(name="a", bufs=1) as apool,
            tc.tile_pool(name="o", bufs=3) as opool,
            tc.tile_pool(name="q", bufs=2) as qpool,
            tc.tile_pool(name="ps_f", bufs=1, space="PSUM") as pfpool,
            tc.tile_pool(name="ps_t", bufs=1, space="PSUM") as ptpool,
            tc.tile_pool(name="ps_i1", bufs=2, space="PSUM") as pi1pool,
            tc.tile_pool(name="ps_i2", bufs=2, space="PSUM") as pi2pool,
        ):
            # ---- constants, ordered by first use
            id_sb = cpool.tile([128, 128], BF16)
            nc.sync.dma_start(out=id_sb[:], in_=id_d[:])
            fw_sb = cpool.tile([128, 1088], BF16)
            nc.sync.dma_start(out=fw_sb[:], in_=fw_d[:])
            ec_sb = cpool.tile([16, 384], BF16)
            nc.sync.dma_start(out=ec_sb[:], in_=ec_d[:])
            f2_sb = cpool.tile([128, 8, 1088], BF16)
            for t in range(8):
                nc.sync.dma_start(out=f2_sb[:, t], in_=f2_d[:, t])
            fc_sb = cpool.tile([128, 384], BF16)
            nc.sync.dma_start(out=fc_sb[:], in_=fc_d[:])
            e1_sb = cpool.tile([128, 64], BF16)
            nc.sync.dma_start(out=e1_sb[:], in_=e1_d[:])
            e2_sb = cpool.tile([128, 64], BF16)
            nc.sync.dma_start(out=e2_sb[:], in_=e2_d[:])
            k2d_sb = cpool.tile([128, 5, 2, 1024], BF16)
            for fcb in range(5):
                nc.sync.dma_start(out=k2d_sb[:, fcb], in_=k2d_d[:, fcb])
            wpk_sb = cpool.tile([128, 4, 3, 2, NF], BF16)
            wh_sb = cpool.tile([128, 2, NF], BF16)

            # ---- W: on-device weight transform -> wpk_sb
            # wpk[128 c, pair, kind, n_in_pair, 544]; kinds: A=wr, B=wr+wi,
            # C=wi-wr (alpha/N normalization folded into fw).
            wt_sb = cpool.tile([128, 128], BF16)
            nc.sync.dma_start(
                out=wt_sb[:],
                in_=xw_d[XROWS:XROWS + WROWS].rearrange(
                    "a (b c) -> (a b) c", b=4, c=256).bitcast(BF16))
            sx_sb = cpool.tile([128, 4], F32)
            nc.sync.dma_start(
                out=sx_sb[:],
                in_=xw_d[XROWS + WROWS:PROWS].rearrange(
                    "a (b c) -> (a b) c", b=64, c=16).bitcast(F32))
            # step 1: spatial DFT (contract k12=81, zero-padded to 128),
            # one [16 cin', 1088] output per filter n so step 2's rhs sits
            # at SBUF partition base 0.
            ws_sb = cpool.tile([16, NFIL, 1088], BF16)
            for n in range(NFIL):
                ws0 = pfpool.tile([128, 512], F32, tag="f0", name=f"ws0_{n}")
                ws1 = pfpool.tile([128, 512], F32, tag="f1", name=f"ws1_{n}")
                ws2 = pfpool.tile([128, 64], F32, tag="f2", name=f"ws2_{n}")
                lw = wt_sb[:, n * 16:(n + 1) * 16]
                nc.tensor.matmul(ws0[0:16, :], lw, fw_sb[:, 0:512],
                                 start=True, stop=True)
                nc.tensor.matmul(ws1[0:16, :], lw, fw_sb[:, 512:1024],
                                 start=True, stop=True)
                nc.tensor.matmul(ws2[0:16, :], lw, fw_sb[:, 1024:1088],
                                 start=True, stop=True)
                nc.scalar.copy(ws_sb[:, n, 0:512], ws0[0:16, :])
                nc.scalar.copy(ws_sb[:, n, 512:1024], ws1[0:16, :])
                nc.scalar.copy(ws_sb[:, n, 1024:1088], ws2[0:16, :])
            # step 2 per n: channel DFT (contract cin'=16) + Gauss pack.
            # Wr = cosT@Wsr + sinT@Wsi ; Wi = cosT@Wsi - sinT@Wsr
            lcos = ec_sb[:, 0:128]
            lsin = ec_sb[:, 128:256]
            lnsin = ec_sb[:, 256:384]
            for n in range(NFIL):
                pr, o = n // 2, n % 2
                re0 = ws_sb[:, n, 0:512]
                re1 = ws_sb[:, n, 512:544]
                im0 = ws_sb[:, n, 544:1056]
                im1 = ws_sb[:, n, 1056:1088]
                wr0 = pfpool.tile([128, 512], F32, tag="f0", name=f"wr0_{n}")
                wi0 = pfpool.tile([128, 512], F32, tag="f1", name=f"wi0_{n}")
                wx1 = pfpool.tile([128, 64], F32, tag="f2", name=f"wx1_{n}")
                nc.tensor.matmul(wr0[:], lcos, re0, start=True, stop=False)
                nc.tensor.matmul(wr0[:], lsin, im0, start=False, stop=True)
                nc.tensor.matmul(wx1[:, 0:32], lcos, re1,
                                 start=True, stop=False, skip_group_check=True)
                nc.tensor.matmul(wx1[:, 0:32], lsin, im1,
                                 start=False, stop=True, skip_group_check=True)
                nc.tensor.matmul(wi0[:], lcos, im0, start=True, stop=False)
                nc.tensor.matmul(wi0[:], lnsin, re0, start=False, stop=True)
                nc.tensor.matmul(wx1[:, 32:64], lcos, im1,
                                 start=True, stop=False, skip_group_check=True)
                nc.tensor.matmul(wx1[:, 32:64], lnsin, re1,
                                 start=False, stop=True, skip_group_check=True)
                # pack: A=wr, B=wr+wi, C=wi-wr (bf16). DVE can't read two
                # PSUM operands, so stage wr/wi through SBUF first.
                nc.scalar.copy(wh_sb[:, 0, 0:512], wr0[:])
                nc.scalar.copy(wh_sb[:, 0, 512:544], wx1[:, 0:32])
                nc.scalar.copy(wh_sb[:, 1, 0:512], wi0[:])
                nc.scalar.copy(wh_sb[:, 1, 512:544], wx1[:, 32:64])
                nc.gpsimd.tensor_scalar_mul(wpk_sb[:, pr, 0, o],
                                            wh_sb[:, 0], 1.0)
                nc.vector.tensor_add(wpk_sb[:, pr, 1, o],
                                     wh_sb[:, 0], wh_sb[:, 1])
                nc.vector.tensor_sub(wpk_sb[:, pr, 2, o],
                                     wh_sb[:, 1], wh_sb[:, 0])

            def emit_forward(b):
                # ---- load u8 x[b] in native [c, hw] layout, dequant-shift
                # to bf16 ints, transpose on PE
                xb8 = xpool.tile([128, 1024], U8, tag="xb8")
                nc.gpsimd.dma_start(
                    out=xb8[:], in_=xw_d[b * 128:(b + 1) * 128, :])
                xb = xpool.tile([128, 1024], BF16, tag="xb")
                nc.vector.tensor_scalar_add(xb[:], xb8[:], -128.0)
                xt = xpool.tile([128, 8, 128], BF16, tag="xt")
                for t in range(8):
                    pt = ptpool.tile([128, 128], BF16, tag="pt")
                    nc.tensor.transpose(
                        pt[:], xb[:, t * 128:(t + 1) * 128], id_sb[:])
                    nc.scalar.copy(xt[:, t, :], pt[:])

                # ---- S: joint real 2D rfft: Xs[c, 1088] in 3 PSUM tiles
                sp0 = pfpool.tile([128, 512], F32, tag="f0", name=f"sp0_{b}")
                sp1 = pfpool.tile([128, 512], F32, tag="f1", name=f"sp1_{b}")
                sp2 = pfpool.tile([128, 64], F32, tag="f2", name=f"sp2_{b}")
                for t in range(8):
                    st = (t == 0)
                    sp = (t == 7)
                    lhsT = xt[:, t, :]
                    nc.tensor.matmul(sp0[:], lhsT, f2_sb[:, t, 0:512],
                                     start=st, stop=sp)
                    nc.tensor.matmul(sp1[:], lhsT, f2_sb[:, t, 512:1024],
                                     start=st, stop=sp)
                    nc.tensor.matmul(sp2[:], lhsT, f2_sb[:, t, 1024:1088],
                                     start=st, stop=sp)
                # fold the per-(b, c) x-dequant scale in while evacuating
                # PSUM (Xs partitions = input channel c)
                sxb = sx_sb[:, b:b + 1]
                xs_sb = xspool.tile([128, 1088], BF16, tag="xs")
                nc.vector.tensor_scalar_mul(xs_sb[:, 0:512], sp0[:], sxb)
                nc.vector.tensor_scalar_mul(xs_sb[:, 512:1024], sp1[:], sxb)
                nc.vector.tensor_scalar_mul(xs_sb[:, 1024:1088], sp2[:], sxb)

                # ---- C: channel DFT -> Xr/Xi [k_c, 544]
                # Xr = cosT@Xsr + sinT@Xsi ; Xi = cosT@Xsi - sinT@Xsr
                cp0 = pfpool.tile([128, 512], F32, tag="f0", name=f"cp0_{b}")
                cp1 = pfpool.tile([128, 512], F32, tag="f1", name=f"cp1_{b}")
                cp2 = pfpool.tile([128, 64], F32, tag="f2", name=f"cp2_{b}")
                ccos = fc_sb[:, 0:128]
                csin = fc_sb[:, 128:256]
                cnsin = fc_sb[:, 256:384]
                nc.tensor.matmul(cp0[:], ccos, xs_sb[:, 0:512],
                                 start=True, stop=False)
                nc.tensor.matmul(cp0[:], csin, xs_sb[:, 544:1056],
                                 start=False, stop=True)
                nc.tensor.matmul(cp2[:, 0:32], ccos, xs_sb[:, 512:544],
                                 start=True, stop=False, skip_group_check=True)
                nc.tensor.matmul(cp2[:, 0:32], csin, xs_sb[:, 1056:1088],
                                 start=False, stop=True, skip_group_check=True)
                nc.tensor.matmul(cp1[:], ccos, xs_sb[:, 544:1056],
                                 start=True, stop=False)
                nc.tensor.matmul(cp1[:], cnsin, xs_sb[:, 0:512],
                                 start=False, stop=True)
                nc.tensor.matmul(cp2[:, 32:64], ccos, xs_sb[:, 1056:1088],
                                 start=True, stop=False, skip_group_check=True)
                nc.tensor.matmul(cp2[:, 32:64], cnsin, xs_sb[:, 512:544],
                                 start=False, stop=True, skip_group_check=True)

                xr = xcpool.tile([128, NF], BF16, tag="xr")
                xi = xcpool.tile([128, NF], BF16, tag="xi")
                xsum = xcpool.tile([128, NF], BF16, tag="xsum")
                nc.vector.tensor_scalar_mul(xr[:, 0:512], cp0[:], 1.0)
                nc.vector.tensor_scalar_mul(xr[:, 512:544], cp2[:, 0:32], 1.0)
                nc.vector.tensor_scalar_mul(xi[:, 0:512], cp1[:], 1.0)
                nc.vector.tensor_scalar_mul(xi[:, 512:544], cp2[:, 32:64], 1.0)
                nc.vector.tensor_add(xsum[:], xr[:], xi[:])

                # ---- MUL per n-pair (z for this b, consumed by back(b))
                zs = []
                for pr in range(4):
                    zr = zpool.tile([128, 2, NFP], BF16, tag=f"zr{pr}", bufs=3)
                    zi = zpool.tile([128, 2, NFP], BF16, tag=f"zi{pr}", bufs=3)
                    k1 = zpool.tile([128, 2, NF], BF16, tag=f"k1{pr}", bufs=2)
                    nc.gpsimd.memset(zr[:, :, NF:NFP].bitcast(F32), 0.0)
                    nc.gpsimd.memset(zi[:, :, NF:NFP].bitcast(F32), 0.0)
                    bc = lambda ap: ap.rearrange(
                        "p (o f) -> p o f", o=1).broadcast_to([128, 2, NF])
                    zrv = zr[:, :, 0:NF]
                    ziv = zi[:, :, 0:NF]
                    # zr = k1 - (wr+wi)*xi ; zi = k1 + (wi-wr)*xr
                    nc.vector.tensor_mul(ziv, bc(xr[:]), wpk_sb[:, pr, 2])
                    nc.vector.tensor_mul(zrv, bc(xi[:]), wpk_sb[:, pr, 1])
                    nc.vector.tensor_mul(k1[:], bc(xsum[:]), wpk_sb[:, pr, 0])
                    eng = nc.gpsimd if pr in POOL_PAIRS else nc.vector
                    eng.tensor_sub(zrv, k1[:], zrv)
                    eng.tensor_add(ziv, k1[:], ziv)
                    zs.append((zr, zi))
                return zs

            def emit_back(b, zs, last=False):
                # ---- I1 per n
                a_sb = apool.tile([128, 5, 2, NFIL, 32], BF16, tag="a")
                for pr in range(4):
                    zr, zi = zs[pr]
                    for o in range(2):
                        n = 2 * pr + o
                        ips = pi1pool.tile([128, 320], F32, tag="ips")
                        for fcb in range(5):
                            col = slice(fcb * 64, (fcb + 1) * 64)
                            zsl = slice(fcb * 128, (fcb + 1) * 128)
                            nc.tensor.matmul(
                                ips[:, col], zr[:, o, zsl], e1_sb[:],
                                start=True, stop=False)
                            nc.tensor.matmul(
                                ips[:, col], zi[:, o, zsl], e2_sb[:],
                                start=False, stop=True)
                        nc.scalar.copy(
                            a_sb[:, :, :, n, :],
                            ips[:].rearrange("p (fc c j) -> p fc c j",
                                             fc=5, c=2))

                # ---- I2: joint 2D inverse + Re extraction + u8 quantization
                for mh in range(2):
                    opsl = []
                    for nzc in range(2):
                        ops = pi2pool.tile([128, 512], F32, tag="psi2")
                        for fcb in range(5):
                            for comp in range(2):
                                lhsT = a_sb[:, fcb, comp,
                                            mh * 4:(mh + 1) * 4, :]
                                rhs = k2d_sb[:, fcb, comp,
                                             nzc * 512:(nzc + 1) * 512]
                                nc.tensor.matmul(
                                    ops[:], lhsT, rhs,
                                    start=(fcb == 0 and comp == 0),
                                    stop=(fcb == 4 and comp == 1))
                        opsl.append(ops)
                    # per-channel (partition) abs-max over the 1024 spatial
                    # values, then inv = QCLIP / absmax
                    am = qpool.tile([128, 4], F32, tag="am")
                    nc.vector.tensor_reduce(
                        am[:, 0:1], opsl[0][:], axis=mybir.AxisListType.X,
                        op=mybir.AluOpType.max, apply_absolute_value=True)
                    nc.vector.tensor_reduce(
                        am[:, 1:2], opsl[1][:], axis=mybir.AxisListType.X,
                        op=mybir.AluOpType.max, apply_absolute_value=True)
                    nc.vector.tensor_max(am[:, 2:3], am[:, 0:1], am[:, 1:2])
                    nc.vector.tensor_scalar_max(am[:, 2:3], am[:, 2:3], 1e-20)
                    inv = qpool.tile([128, 1], F32, tag="inv")
                    nc.vector.reciprocal(inv[:], am[:, 2:3])
                    nc.vector.tensor_scalar_mul(inv[:], inv[:], QCLIP)
                    # quantize both 512-chunks: u8 = ops * inv + QOFF
                    o_sb = opool.tile([128, 1024], U8, tag="o")
                    nc.vector.tensor_scalar(
                        o_sb[:, 0:512], opsl[0][:], inv[:], QOFF,
                        op0=mybir.AluOpType.mult, op1=mybir.AluOpType.add)
                    nc.vector.tensor_scalar(
                        o_sb[:, 512:1024], opsl[1][:], inv[:], QOFF,
                        op0=mybir.AluOpType.mult, op1=mybir.AluOpType.add)
                    nc.sync.dma_start(
                        out=out_d[b, mh * 128:(mh + 1) * 128, :],
                        in_=o_sb[:])
                    # ship the f32 inv factors in row 256
                    inv_dst = out_d[b, 256].bitcast(F32).rearrange(
                        "(p one) -> p one", one=1)[mh * 128:(mh + 1) * 128]
                    nc.sync.dma_start(out=inv_dst, in_=inv[:])

            # ---- software-pipelined schedule, depth 1:
            # fwd(b) runs one PE window ahead of back(b)
            zs_all = {}
            DEPTH = 1
            for b in range(B_LOC):
                zs_all[b] = emit_forward(b)
                if b >= DEPTH:
                    emit_back(b - DEPTH, zs_all[b - DEPTH])
            for b in range(B_LOC - DEPTH, B_LOC):
                emit_back(b, zs_all[b], last=(b == B_LOC - 1))
    nc.compile()
    return nc


# ----------------------------------------------------------------- host side
_CACHE = {}


def _setup():
    """Build program + cached jit + device-resident consts. Runs once."""
    import jax
    import jax.numpy as jnp  # noqa: F401
    from jax.sharding import Mesh, PartitionSpec, NamedSharding
    from jax.experimental.shard_map import shard_map
    from concourse._compat import axon_active

    try:
        # persistent executable cache: makes the first call in a fresh
        # process load the compiled NEFF-wrapped executable from disk
        # instead of recompiling (~1-3 min). Harmless no-op on miss.
        jax.config.update("jax_compilation_cache_dir",
                          "/root/.cache/jax_bass_cc")
        jax.config.update("jax_persistent_cache_min_compile_time_secs", 2.0)
    except Exception:
        pass

    nc = build_program()
    if not axon_active():
        # native /dev/neuron* path: no bass_exec custom-call executor, so
        # run through run_bass_kernel_spmd (slower: re-jits + re-uploads
        # consts per call, but correct anywhere)
        _CACHE.update(dict(nc=nc, consts=build_consts(), fallback=True))
        return _CACHE

    bass2jax.install_neuronx_cc_hook()

    partition_name = (nc.partition_id_tensor.name
                      if nc.partition_id_tensor else None)
    in_names = []
    out_names = []
    out_avals = []
    for alloc in nc.m.functions[0].allocations:
        if not isinstance(alloc, mybir.MemoryLocationSet):
            continue
        name = alloc.memorylocations[0].name
        if alloc.kind == "ExternalInput":
            if name != partition_name:
                in_names.append(name)
        elif alloc.kind == "ExternalOutput":
            out_names.append(name)
            out_avals.append(jax.core.ShapedArray(
                tuple(alloc.tensor_shape), mybir.dt.np(alloc.dtype)))
    all_in_names = list(in_names) + list(out_names)
    if partition_name is not None:
        all_in_names.append(partition_name)

    dbg_extra = {}
    if nc.dbg_addr is not None:
        dbg_extra[nc.dbg_addr.name] = np.zeros((1, 2), np.uint32)

    def _body(*args):
        operands = list(args)
        if partition_name is not None:
            operands.append(bass2jax.partition_id_tensor())
        outs = bass2jax._bass_exec_p.bind(
            *operands,
            out_avals=tuple(out_avals),
            in_names=tuple(all_in_names),
            out_names=tuple(out_names),
            lowering_input_output_aliases=(),
            sim_require_finite=True,
            sim_require_nnan=True,
            nc=nc,
        )
        return tuple(outs)

    devices = jax.devices()[:N_CORES]
    mesh = Mesh(np.asarray(devices), ("core",))
    n_args = len(in_names) + len(out_names)
    sharded = jax.jit(shard_map(
        _body, mesh=mesh,
        in_specs=(PartitionSpec("core"),) * n_args,
        out_specs=(PartitionSpec("core"),) * len(out_names),
        check_rep=False))
    shard = NamedSharding(mesh, PartitionSpec("core"))

    # device-resident weight-independent inputs: consts (replicated per
    # core) + zero output buffers (never donated, so they persist).
    consts = build_consts()
    static = {}
    for name in in_names:
        if name == "xw":
            continue
        if name in consts:
            a = consts[name]
        elif nc.dbg_addr is not None and name == nc.dbg_addr.name:
            a = dbg_extra[name]
        else:
            raise KeyError(f"unexpected input {name}")
        tiled = np.ascontiguousarray(
            np.broadcast_to(a, (N_CORES,) + a.shape).reshape(
                (N_CORES * a.shape[0],) + a.shape[1:]))
        static[name] = jax.device_put(tiled, shard)
    zero_bufs = {}
    for name, av in zip(out_names, out_avals):
        zero_bufs[name] = jax.device_put(
            np.zeros((N_CORES * av.shape[0],) + av.shape[1:], av.dtype),
            shard)
    for v in static.values():
        v.block_until_ready()
    for v in zero_bufs.values():
        v.block_until_ready()

    _CACHE.update(dict(nc=nc, sharded=sharded, shard=shard,
                       in_names=in_names, out_names=out_names,
                       static=static, zero_bufs=zero_bufs, jax=jax))
    return _CACHE


def _pack_payload(x, weight):
    """Pack u8-quantized x + weight + dequant scales into the payload."""
    x3 = x.reshape(N_CORES, B_LOC * CIN, L * L)
    xw = np.empty((N_CORES, PROWS, 1024), dtype=np.uint8)
    amax = np.maximum(np.maximum(x3.max(axis=2), -x3.min(axis=2)), 1e-20)
    tmp = _CACHE.get("pack_tmp")
    if tmp is None:
        tmp = _CACHE["pack_tmp"] = np.empty_like(x3)
    np.multiply(x3, (XQ / amax)[:, :, None], out=tmp)
    np.add(tmp, 128.5, out=tmp)
    # u8 cast truncates: floor(v + 128.5) == round(v) + 128, range [1, 255]
    np.copyto(xw[:, :XROWS, :], tmp, casting="unsafe")
    # wT[k1*9+k2, n*16+cin'] zero-padded to [128, 128] bf16, as raw bytes
    wt = np.zeros((128, 128), dtype=NPBF)
    wt[:81] = weight.transpose(2, 3, 0, 1).reshape(81, 128)
    xw[:, XROWS:XROWS + WROWS, :] = wt.view(np.uint8).reshape(32, 1024)
    # f32 dequant scales, laid out [c, b] per core
    sxc = np.ascontiguousarray(
        (amax / XQ).astype(np.float32).reshape(
            N_CORES, B_LOC, CIN).transpose(0, 2, 1))       # [8, 128, 4]
    xw[:, XROWS + WROWS:PROWS, :] = sxc.view(np.uint8).reshape(
        N_CORES, SROWS, 1024)
    return xw.reshape(N_CORES * PROWS, 1024)


def _dequant(raw):
    """u8 [B, 257, 1024] device result -> f32 [B, 256, 32, 32]."""
    inv = np.ascontiguousarray(raw[:, 256, :]).view(np.float32)  # [B, 256]
    scale = (1.0 / inv.astype(np.float64)).astype(np.float32)
    out = np.empty((B, 256, 1024), np.float32)
    np.subtract(raw[:, :256, :], np.float32(QOFF), out=out, casting="unsafe")
    np.multiply(out, scale[:, :, None], out=out)
    return out.reshape(B, 256, 32, 32)


def kernel(x, weight):
    x = np.asarray(x, dtype=np.float32)
    weight = np.asarray(weight, dtype=np.float32)
    if "sharded" not in _CACHE and "fallback" not in _CACHE:
        _setup()
    c = _CACHE

    if c.get("fallback"):
        from concourse.bass_utils import run_bass_kernel_spmd
        xw = _pack_payload(x, weight).reshape(N_CORES, PROWS, 1024)
        in_maps = []
        for i in range(N_CORES):
            m = {"xw": xw[i]}
            m.update(c["consts"])
            in_maps.append(m)
        res = run_bass_kernel_spmd(c["nc"], in_maps,
                                   core_ids=list(range(N_CORES)))
        raw = np.concatenate([r["out"] for r in res.results], axis=0)
        return _dequant(raw)

    jax = c["jax"]
    xw = _pack_payload(x, weight)
    xw_dev = jax.device_put(xw, c["shard"])
    args = []
    for name in c["in_names"]:
        args.append(xw_dev if name == "xw" else c["static"][name])
    for name in c["out_names"]:
        args.append(c["zero_bufs"][name])
    outs = c["sharded"](*args)
    raw = np.asarray(outs[0])                       # u8 [B, 257, 1024]
    _CACHE["last_raw"] = raw
    return _dequant(raw)


if __name__ == "__main__":
    import jax

    sys.path.insert(0, "/root/problem")
    from reference import setup_inputs, reference

    with jax.default_device(jax.devices("cpu")[0]):
        inputs = setup_inputs()
        inputs = {k: np.asarray(v) for k, v in inputs.items()}
        expected = np.asarray(reference(**inputs))
    actual = kernel(**inputs)
    err = np.linalg.norm(actual - expected) / np.linalg.norm(expected)
    print("Relative error:", err)


# revision 31
# speedup vs baseline: 1.0065x; 1.0065x over previous
"""Trainium2 Bass kernel for FConv2d (FFT conv module), v5.

out = irfftn( rfftn(x, axes=(c,h,w)) * rfftn(pad(weight)) )[:, :, ::4] reshaped.

Device math (data-parallel over batch, 4 per core x 8 cores), bf16 data with
fp32 PSUM accumulation:
  W   on-device weight transform: spatial DFT of the 9x9 taps (contract 81)
      then channel DFT (contract 16), Gauss-packed into wpk for the MUL stage.
  T   on-device u8->bf16 shift + transpose of x[b] ([c, hw] -> [hw, c]
      chunks) via PE identity matmuls; host ships x in its native layout.
  S   joint 2D spatial rFFT while data is REAL: contract hw=1024 via 8
      PSUM-accumulated matmuls per batch; the per-(b, c) x-dequant scale is
      folded into the PSUM evacuation.
  C   channel DFT (contract c=128) producing X[k_c, f] for 544 spatial freqs.
  MUL complex multiply with W-hat via Gauss 3-mult on DVE (+Pool offload).
  I1  subsampled inverse channel DFT via paired matmuls.
  I2  joint 2D spatial inverse + Re extraction via paired matmuls, then
      per-channel abs-max u8 quantization of the output on device.

The wall-clock bottleneck is host<->device traffic: the NeuronCores sit
behind an axon tunnel moving ~35-50 MB/s with ~80 ms per-call dispatch
latency, while the device compute itself is ~0.2 ms. Per call this kernel
ships one 4.5 MB H2D payload (u8-quantized x + bf16 weight bytes + f32
scales) and one 8.9 MB D2H result (u8-quantized out + f32 per-channel
scales, dequantized on host). All DFT twiddle constants and the output
zero buffers are weight-independent, device-resident, and uploaded only
once; the jitted shard_map executable is cached across calls.
"""
import sys
import numpy as np

for _p in ("/opt/trn_rl_repo", "/root/.axon_site/_ro/trn_rl_repo"):
    if _p not in sys.path:
        sys.path.insert(0, _p)

import ml_dtypes

import concourse.bacc as bacc
import concourse.bass as bass
import concourse.mybir as mybir
import concourse.tile as tile
from concourse import bass2jax

F32 = mybir.dt.float32
BF16 = mybir.dt.bfloat16
U8 = mybir.dt.uint8
NPBF = ml_dtypes.bfloat16

# u8 output quantization: u = out * (QCLIP/absmax) + QOFF on device (DVE
# float->u8 convert), out = (u - QOFF) / inv on host. QCLIP < 127 leaves
# headroom so saturation/rounding at the extremes cannot wrap.
QCLIP = 126.5
QOFF = 128.0

B = 32          # full batch
B_LOC = 4       # per core
N_CORES = 8
CIN = 128
L = 32
NFIL = 8        # num filters n
NF = 544        # stored spatial freqs (32 * 17)
NFP = 640       # padded: 5 chunks of 128
XROWS = B_LOC * CIN          # 512 rows of u8-quantized x per core
WROWS = 32                   # 32 rows holding the packed [128,128] bf16 weight
SROWS = 2                    # 2 rows holding the 512 f32 dequant scales
PROWS = XROWS + WROWS + SROWS   # 546 payload rows per core

# x u8 quantization: q = round(x * 127/absmax) + 128 per (b, c) row on host;
# device subtracts 128 during the u8->bf16 convert and folds absmax/127 into
# the spatial-rFFT output (per-partition scale).
XQ = 127.0

# which n-pairs get their zr/zi add/sub on Pool instead of DVE
POOL_PAIRS = (0, 1, 2, 3)


# ----------------------------------------------------------------- constants
def build_consts():
    """Weight-independent DFT factor tensors (bf16). Uploaded once."""
    f = np.arange(NF)
    p = f // 17
    q = f % 17

    # S: joint real 2D rfft factor, per hw chunk t: [128 hw, 1088]
    # cols = [Re(f) 0:544 | Im(f) 544:1088];  Re = cos, Im = -sin
    f2 = np.zeros((128, 8, 1088), dtype=np.float64)
    hw_p = np.arange(128)
    for t in range(8):
        h = 4 * t + hw_p // 32
        w = hw_p % 32
        ang = 2 * np.pi * (np.outer(h, p) + np.outer(w, q)) / 32.0
        f2[:, t, 0:NF] = np.cos(ang)
        f2[:, t, NF:2 * NF] = -np.sin(ang)

    # C: channel DFT lhsT pack [c, 384] = [cos | sin | -sin]
    c = np.arange(128)
    k = np.arange(128)
    angc = 2 * np.pi * np.outer(c, k) / 128.0
    fc = np.concatenate(
        [np.cos(angc), np.sin(angc), -np.sin(angc)], axis=1)

    # I1 rhs: e1 = [cos|sin], e2 = [-sin|cos] at output positions c=4j
    j32 = np.arange(32)
    ange = 2 * np.pi * np.outer(k, j32) / 32.0
    er = np.cos(ange)
    ei = np.sin(ange)
    e1 = np.concatenate([er, ei], axis=1)
    e2 = np.concatenate([-ei, er], axis=1)

    # I2 rhs: k2d[128, 5, 2, 1024]: cos / -sin of inverse angles
    k2d = np.zeros((128, 5, 2, 1024), dtype=np.float64)
    yz = np.arange(1024)
    y = yz // 32
    z = yz % 32
    fp = np.arange(NFP)
    valid = (fp < NF).astype(np.float64)
    pv = np.zeros(NFP, dtype=np.int64)
    qv = np.zeros(NFP, dtype=np.int64)
    pv[:NF] = p
    qv[:NF] = q
    for fcb in range(5):
        sl = slice(fcb * 128, (fcb + 1) * 128)
        ang4 = 2 * np.pi * (np.outer(pv[sl], y) + np.outer(qv[sl], z)) / 32.0
        k2d[:, fcb, 0, :] = np.cos(ang4) * valid[sl][:, None]
        k2d[:, fcb, 1, :] = -np.sin(ang4) * valid[sl][:, None]

    # W stage step 1: spatial DFT of the 9x9 taps; rows = k1*9+k2 (pad to
    # 128), cols = [Re 544 | Im 544], with alpha[q]/(128*32*32) folded in.
    alpha = np.full(17, 2.0)
    alpha[0] = 1.0
    alpha[16] = 1.0
    scale = alpha[q] / (128.0 * 32.0 * 32.0)
    k12 = np.arange(81)
    k1 = k12 // 9
    k2 = k12 % 9
    angw = 2 * np.pi * (np.outer(k1, p) + np.outer(k2, q)) / 32.0
    fw = np.zeros((128, 1088), dtype=np.float64)
    fw[:81, 0:NF] = np.cos(angw) * scale
    fw[:81, NF:2 * NF] = -np.sin(angw) * scale

    # W stage step 2: channel DFT lhsT pack over cin'=16: [16, 384] =
    # [cos | sin | -sin] of 2*pi*c*cin'/128
    ci = np.arange(16)
    angec = 2 * np.pi * np.outer(ci, c) / 128.0
    ec = np.concatenate(
        [np.cos(angec), np.sin(angec), -np.sin(angec)], axis=1)

    # T stage: identity for PE transposes
    id128 = np.eye(128)

    cast = lambda a: np.ascontiguousarray(a).astype(NPBF)
    return {
        "f2": cast(f2),
        "fc": cast(fc),
        "e1": cast(e1),
        "e2": cast(e2),
        "k2d": cast(k2d),
        "fw": cast(fw),
        "ec": cast(ec),
        "id128": cast(id128),
    }


# ----------------------------------------------------------------- program
def build_program():
    nc = bacc.Bacc("TRN2", target_bir_lowering=False, debug=False)
    # xw rows 0:512 = u8-quantized x[b] as [c, h*w] (native layout); rows
    # 512:544 = weight packed [128, 128] bf16 (wT[k1*9+k2, n*16+cin'],
    # zero-padded) as raw bytes; rows 544:546 = the 512 f32 x-dequant scales
    # laid out [c, b].
    xw_d = nc.dram_tensor("xw", [PROWS, 1024], U8, kind="ExternalInput")
    f2_d = nc.dram_tensor("f2", [128, 8, 1088], BF16, kind="ExternalInput")
    fc_d = nc.dram_tensor("fc", [128, 384], BF16, kind="ExternalInput")
    e1_d = nc.dram_tensor("e1", [128, 64], BF16, kind="ExternalInput")
    e2_d = nc.dram_tensor("e2", [128, 64], BF16, kind="ExternalInput")
    k2d_d = nc.dram_tensor("k2d", [128, 5, 2, 1024], BF16, kind="ExternalInput")
    fw_d = nc.dram_tensor("fw", [128, 1088], BF16, kind="ExternalInput")
    ec_d = nc.dram_tensor("ec", [16, 384], BF16, kind="ExternalInput")
    id_d = nc.dram_tensor("id128", [128, 128], BF16, kind="ExternalInput")
    # rows 0:256 = u8-quantized out channels; row 256 = the 256 per-channel
    # f32 inverse scales (bitcast into the u8 row).
    out_d = nc.dram_tensor("out", [B_LOC, 257, 1024], U8,
                           kind="ExternalOutput")

    with tile.TileContext(nc) as tc:
        with (
            tc.tile_pool(name="consts", bufs=1) as cpool,
            tc.tile_pool(name="xin", bufs=2) as xpool,
            tc.tile_pool(name="xs", bufs=2) as xspool,
            tc.tile_pool(name="xc", bufs=2) as xcpool,
            tc.tile_pool(name="z", bufs=2) as zpool,
            tc.tile_pool(name="a", bufs=1) as apool,
            tc.tile_pool(name="o", bufs=3) as opool,
            tc.tile_pool(name="q", bufs=2) as qpool,
            tc.tile_pool(name="ps_f", bufs=1, space="PSUM") as pfpool,
            tc.tile_pool(name="ps_t", bufs=1, space="PSUM") as ptpool,
            tc.tile_pool(name="ps_i1", bufs=2, space="PSUM") as pi1pool,
            tc.tile_pool(name="ps_i2", bufs=2, space="PSUM") as pi2pool,
        ):
            # ---- constants, ordered by first use
            id_sb = cpool.tile([128, 128], BF16)
            nc.sync.dma_start(out=id_sb[:], in_=id_d[:])
            fw_sb = cpool.tile([128, 1088], BF16)
            nc.sync.dma_start(out=fw_sb[:], in_=fw_d[:])
            ec_sb = cpool.tile([16, 384], BF16)
            nc.sync.dma_start(out=ec_sb[:], in_=ec_d[:])
            f2_sb = cpool.tile([128, 8, 1088], BF16)
            for t in range(8):
                nc.sync.dma_start(out=f2_sb[:, t], in_=f2_d[:, t])
            fc_sb = cpool.tile([128, 384], BF16)
            nc.sync.dma_start(out=fc_sb[:], in_=fc_d[:])
            e1_sb = cpool.tile([128, 64], BF16)
            nc.sync.dma_start(out=e1_sb[:], in_=e1_d[:])
            e2_sb = cpool.tile([128, 64], BF16)
            nc.sync.dma_start(out=e2_sb[:], in_=e2_d[:])
            k2d_sb = cpool.tile([128, 5, 2, 1024], BF16)
            for fcb in range(5):
                nc.sync.dma_start(out=k2d_sb[:, fcb], in_=k2d_d[:, fcb])
            wpk_sb = cpool.tile([128, 4, 3, 2, NF], BF16)
            wh_sb = cpool.tile([128, 2, NF], BF16)

            # ---- W: on-device weight transform -> wpk_sb
            # wpk[128 c, pair, kind, n_in_pair, 544]; kinds: A=wr, B=wr+wi,
            # C=wi-wr (alpha/N normalization folded into fw).
            wt_sb = cpool.tile([128, 128], BF16)
            nc.sync.dma_start(
                out=wt_sb[:],
                in_=xw_d[XROWS:XROWS + WROWS].rearrange(
                    "a (b c) -> (a b) c", b=4, c=256).bitcast(BF16))
            sx_sb = cpool.tile([128, 4], F32)
            nc.sync.dma_start(
                out=sx_sb[:],
                in_=xw_d[XROWS + WROWS:PROWS].rearrange(
                    "a (b c) -> (a b) c", b=64, c=16).bitcast(F32))
            # step 1: spatial DFT (contract k12=81, zero-padded to 128),
            # one [16 cin', 1088] output per filter n so step 2's rhs sits
            # at SBUF partition base 0.
            ws_sb = cpool.tile([16, NFIL, 1088], BF16)
            for n in range(NFIL):
                ws0 = pfpool.tile([128, 512], F32, tag="f0", name=f"ws0_{n}")
                ws1 = pfpool.tile([128, 512], F32, tag="f1", name=f"ws1_{n}")
                ws2 = pfpool.tile([128, 64], F32, tag="f2", name=f"ws2_{n}")
                lw = wt_sb[:, n * 16:(n + 1) * 16]
                nc.tensor.matmul(ws0[0:16, :], lw, fw_sb[:, 0:512],
                                 start=True, stop=True)
                nc.tensor.matmul(ws1[0:16, :], lw, fw_sb[:, 512:1024],
                                 start=True, stop=True)
                nc.tensor.matmul(ws2[0:16, :], lw, fw_sb[:, 1024:1088],
                                 start=True, stop=True)
                nc.scalar.copy(ws_sb[:, n, 0:512], ws0[0:16, :])
                nc.scalar.copy(ws_sb[:, n, 512:1024], ws1[0:16, :])
                nc.scalar.copy(ws_sb[:, n, 1024:1088], ws2[0:16, :])
            # step 2 per n: channel DFT (contract cin'=16) + Gauss pack.
            # Wr = cosT@Wsr + sinT@Wsi ; Wi = cosT@Wsi - sinT@Wsr
            lcos = ec_sb[:, 0:128]
            lsin = ec_sb[:, 128:256]
            lnsin = ec_sb[:, 256:384]
            for n in range(NFIL):
                pr, o = n // 2, n % 2
                re0 = ws_sb[:, n, 0:512]
                re1 = ws_sb[:, n, 512:544]
                im0 = ws_sb[:, n, 544:1056]
                im1 = ws_sb[:, n, 1056:1088]
                wr0 = pfpool.tile([128, 512], F32, tag="f0", name=f"wr0_{n}")
                wi0 = pfpool.tile([128, 512], F32, tag="f1", name=f"wi0_{n}")
                wx1 = pfpool.tile([128, 64], F32, tag="f2", name=f"wx1_{n}")
                nc.tensor.matmul(wr0[:], lcos, re0, start=True, stop=False)
                nc.tensor.matmul(wr0[:], lsin, im0, start=False, stop=True)
                nc.tensor.matmul(wx1[:, 0:32], lcos, re1,
                                 start=True, stop=False, skip_group_check=True)
                nc.tensor.matmul(wx1[:, 0:32], lsin, im1,
                                 start=False, stop=True, skip_group_check=True)
                nc.tensor.matmul(wi0[:], lcos, im0, start=True, stop=False)
                nc.tensor.matmul(wi0[:], lnsin, re0, start=False, stop=True)
                nc.tensor.matmul(wx1[:, 32:64], lcos, im1,
                                 start=True, stop=False, skip_group_check=True)
                nc.tensor.matmul(wx1[:, 32:64], lnsin, re1,
                                 start=False, stop=True, skip_group_check=True)
                # pack: A=wr, B=wr+wi, C=wi-wr (bf16). DVE can't read two
                # PSUM operands, so stage wr/wi through SBUF first.
                nc.scalar.copy(wh_sb[:, 0, 0:512], wr0[:])
                nc.scalar.copy(wh_sb[:, 0, 512:544], wx1[:, 0:32])
                nc.scalar.copy(wh_sb[:, 1, 0:512], wi0[:])
                nc.scalar.copy(wh_sb[:, 1, 512:544], wx1[:, 32:64])
                nc.gpsimd.tensor_scalar_mul(wpk_sb[:, pr, 0, o],
                                            wh_sb[:, 0], 1.0)
                nc.vector.tensor_add(wpk_sb[:, pr, 1, o],
                                     wh_sb[:, 0], wh_sb[:, 1])
                nc.vector.tensor_sub(wpk_sb[:, pr, 2, o],
                                     wh_sb[:, 1], wh_sb[:, 0])

            def emit_forward(b):
                # ---- load u8 x[b] in native [c, hw] layout, dequant-shift
                # to bf16 ints, transpose on PE
                xb8 = xpool.tile([128, 1024], U8, tag="xb8")
                nc.gpsimd.dma_start(
                    out=xb8[:], in_=xw_d[b * 128:(b + 1) * 128, :])
                xb = xpool.tile([128, 1024], BF16, tag="xb")
                nc.vector.tensor_scalar_add(xb[:], xb8[:], -128.0)
                xt = xpool.tile([128, 8, 128], BF16, tag="xt")
                for t in range(8):
                    pt = ptpool.tile([128, 128], BF16, tag="pt")
                    nc.tensor.transpose(
                        pt[:], xb[:, t * 128:(t + 1) * 128], id_sb[:])
                    nc.scalar.copy(xt[:, t, :], pt[:])

                # ---- S: joint real 2D rfft: Xs[c, 1088] in 3 PSUM tiles
                sp0 = pfpool.tile([128, 512], F32, tag="f0", name=f"sp0_{b}")
                sp1 = pfpool.tile([128, 512], F32, tag="f1", name=f"sp1_{b}")
                sp2 = pfpool.tile([128, 64], F32, tag="f2", name=f"sp2_{b}")
                for t in range(8):
                    st = (t == 0)
                    sp = (t == 7)
                    lhsT = xt[:, t, :]
                    nc.tensor.matmul(sp0[:], lhsT, f2_sb[:, t, 0:512],
                                     start=st, stop=sp)
                    nc.tensor.matmul(sp1[:], lhsT, f2_sb[:, t, 512:1024],
                                     start=st, stop=sp)
                    nc.tensor.matmul(sp2[:], lhsT, f2_sb[:, t, 1024:1088],
                                     start=st, stop=sp)
                # fold the per-(b, c) x-dequant scale in while evacuating
                # PSUM (Xs partitions = input channel c)
                sxb = sx_sb[:, b:b + 1]
                xs_sb = xspool.tile([128, 1088], BF16, tag="xs")
                nc.vector.tensor_scalar_mul(xs_sb[:, 0:512], sp0[:], sxb)
                nc.vector.tensor_scalar_mul(xs_sb[:, 512:1024], sp1[:], sxb)
                nc.vector.tensor_scalar_mul(xs_sb[:, 1024:1088], sp2[:], sxb)

                # ---- C: channel DFT -> Xr/Xi [k_c, 544]
                # Xr = cosT@Xsr + sinT@Xsi ; Xi = cosT@Xsi - sinT@Xsr
                cp0 = pfpool.tile([128, 512], F32, tag="f0", name=f"cp0_{b}")
                cp1 = pfpool.tile([128, 512], F32, tag="f1", name=f"cp1_{b}")
                cp2 = pfpool.tile([128, 64], F32, tag="f2", name=f"cp2_{b}")
                ccos = fc_sb[:, 0:128]
                csin = fc_sb[:, 128:256]
                cnsin = fc_sb[:, 256:384]
                nc.tensor.matmul(cp0[:], ccos, xs_sb[:, 0:512],
                                 start=True, stop=False)
                nc.tensor.matmul(cp0[:], csin, xs_sb[:, 544:1056],
                                 start=False, stop=True)
                nc.tensor.matmul(cp2[:, 0:32], ccos, xs_sb[:, 512:544],
                                 start=True, stop=False, skip_group_check=True)
                nc.tensor.matmul(cp2[:, 0:32], csin, xs_sb[:, 1056:1088],
                                 start=False, stop=True, skip_group_check=True)
                nc.tensor.matmul(cp1[:], ccos, xs_sb[:, 544:1056],
                                 start=True, stop=False)
                nc.tensor.matmul(cp1[:], cnsin, xs_sb[:, 0:512],
                                 start=False, stop=True)
                nc.tensor.matmul(cp2[:, 32:64], ccos, xs_sb[:, 1056:1088],
                                 start=True, stop=False, skip_group_check=True)
                nc.tensor.matmul(cp2[:, 32:64], cnsin, xs_sb[:, 512:544],
                                 start=False, stop=True, skip_group_check=True)

                xr = xcpool.tile([128, NF], BF16, tag="xr")
                xi = xcpool.tile([128, NF], BF16, tag="xi")
                xsum = xcpool.tile([128, NF], BF16, tag="xsum")
                nc.vector.tensor_scalar_mul(xr[:, 0:512], cp0[:], 1.0)
                nc.vector.tensor_scalar_mul(xr[:, 512:544], cp2[:, 0:32], 1.0)
                nc.vector.tensor_scalar_mul(xi[:, 0:512], cp1[:], 1.0)
                nc.vector.tensor_scalar_mul(xi[:, 512:544], cp2[:, 32:64], 1.0)
                nc.vector.tensor_add(xsum[:], xr[:], xi[:])

                # ---- MUL per n-pair (z for this b, consumed by back(b))
                zs = []
                for pr in range(4):
                    zr = zpool.tile([128, 2, NFP], BF16, tag=f"zr{pr}", bufs=3)
                    zi = zpool.tile([128, 2, NFP], BF16, tag=f"zi{pr}", bufs=3)
                    k1 = zpool.tile([128, 2, NF], BF16, tag=f"k1{pr}", bufs=2)
                    nc.gpsimd.memset(zr[:, :, NF:NFP].bitcast(F32), 0.0)
                    nc.gpsimd.memset(zi[:, :, NF:NFP].bitcast(F32), 0.0)
                    bc = lambda ap: ap.rearrange(
                        "p (o f) -> p o f", o=1).broadcast_to([128, 2, NF])
                    zrv = zr[:, :, 0:NF]
                    ziv = zi[:, :, 0:NF]
                    # zr = k1 - (wr+wi)*xi ; zi = k1 + (wi-wr)*xr
                    nc.vector.tensor_mul(ziv, bc(xr[:]), wpk_sb[:, pr, 2])
                    nc.vector.tensor_mul(zrv, bc(xi[:]), wpk_sb[:, pr, 1])
                    nc.vector.tensor_mul(k1[:], bc(xsum[:]), wpk_sb[:, pr, 0])
                    eng = nc.gpsimd if pr in POOL_PAIRS else nc.vector
                    eng.tensor_sub(zrv, k1[:], zrv)
                    eng.tensor_add(ziv, k1[:], ziv)
                    zs.append((zr, zi))
                return zs

            def emit_back(b, zs, last=False):
                # ---- I1 per n
                a_sb = apool.tile([128, 5, 2, NFIL, 32], BF16, tag="a")
                for pr in range(4):
                    zr, zi = zs[pr]
                    for o in range(2):
                        n = 2 * pr + o
                        ips = pi1pool.tile([128, 320], F32, tag="ips")
                        for fcb in range(5):
                            col = slice(fcb * 64, (fcb + 1) * 64)
                            zsl = slice(fcb * 128, (fcb + 1) * 128)
                            nc.tensor.matmul(
                                ips[:, col], zr[:, o, zsl], e1_sb[:],
                                start=True, stop=False)
                            nc.tensor.matmul(
                                ips[:, col], zi[:, o, zsl], e2_sb[:],
                                start=False, stop=True)
                        nc.scalar.copy(
                            a_sb[:, :, :, n, :],
                            ips[:].rearrange("p (fc c j) -> p fc c j",
                                             fc=5, c=2))

                # ---- I2: joint 2D inverse + Re extraction + u8 quantization
                for mh in range(2):
                    opsl = []
                    for nzc in range(2):
                        ops = pi2pool.tile([128, 512], F32, tag="psi2")
                        for fcb in range(5):
                            for comp in range(2):
                                lhsT = a_sb[:, fcb, comp,
                                            mh * 4:(mh + 1) * 4, :]
                                rhs = k2d_sb[:, fcb, comp,
                                             nzc * 512:(nzc + 1) * 512]
                                nc.tensor.matmul(
                                    ops[:], lhsT, rhs,
                                    start=(fcb == 0 and comp == 0),
                                    stop=(fcb == 4 and comp == 1))
                        opsl.append(ops)
                    # per-channel (partition) abs-max over the 1024 spatial
                    # values, then inv = QCLIP / absmax
                    am = qpool.tile([128, 4], F32, tag="am")
                    nc.vector.tensor_reduce(
                        am[:, 0:1], opsl[0][:], axis=mybir.AxisListType.X,
                        op=mybir.AluOpType.max, apply_absolute_value=True)
                    nc.vector.tensor_reduce(
                        am[:, 1:2], opsl[1][:], axis=mybir.AxisListType.X,
                        op=mybir.AluOpType.max, apply_absolute_value=True)
                    nc.vector.tensor_max(am[:, 2:3], am[:, 0:1], am[:, 1:2])
                    nc.vector.tensor_scalar_max(am[:, 2:3], am[:, 2:3], 1e-20)
                    inv = qpool.tile([128, 1], F32, tag="inv")
                    nc.vector.reciprocal(inv[:], am[:, 2:3])
                    nc.vector.tensor_scalar_mul(inv[:], inv[:], QCLIP)
                    # quantize both 512-chunks: u8 = ops * inv + QOFF
                    o_sb = opool.tile([128, 1024], U8, tag="o")
                    nc.vector.tensor_scalar(
                        o_sb[:, 0:512], opsl[0][:], inv[:], QOFF,
                        op0=mybir.AluOpType.mult, op1=mybir.AluOpType.add)
                    nc.vector.tensor_scalar(
                        o_sb[:, 512:1024], opsl[1][:], inv[:], QOFF,
                        op0=mybir.AluOpType.mult, op1=mybir.AluOpType.add)
                    nc.sync.dma_start(
                        out=out_d[b, mh * 128:(mh + 1) * 128, :],
                        in_=o_sb[:])
                    # ship the f32 inv factors in row 256
                    inv_dst = out_d[b, 256].bitcast(F32).rearrange(
                        "(p one) -> p one", one=1)[mh * 128:(mh + 1) * 128]
                    nc.sync.dma_start(out=inv_dst, in_=inv[:])

            # ---- software-pipelined schedule, depth 1:
            # fwd(b) runs one PE window ahead of back(b)
            zs_all = {}
            DEPTH = 1
            for b in range(B_LOC):
                zs_all[b] = emit_forward(b)
                if b >= DEPTH:
                    emit_back(b - DEPTH, zs_all[b - DEPTH])
            for b in range(B_LOC - DEPTH, B_LOC):
                emit_back(b, zs_all[b], last=(b == B_LOC - 1))
    nc.compile()
    return nc


# ----------------------------------------------------------------- host side
_CACHE = {}


def _setup():
    """Build program + cached jit + device-resident consts. Runs once."""
    import jax
    import jax.numpy as jnp  # noqa: F401
    from jax.sharding import Mesh, PartitionSpec, NamedSharding
    from jax.experimental.shard_map import shard_map
    from concourse._compat import axon_active

    try:
        # persistent executable cache: makes the first call in a fresh
        # process load the compiled NEFF-wrapped executable from disk
        # instead of recompiling (~1-3 min). Harmless no-op on miss.
        jax.config.update("jax_compilation_cache_dir",
                          "/root/.cache/jax_bass_cc")
        jax.config.update("jax_persistent_cache_min_compile_time_secs", 2.0)
    except Exception:
        pass

    nc = build_program()
    if not axon_active():
        # native /dev/neuron* path: no bass_exec custom-call executor, so
        # run through run_bass_kernel_spmd (slower: re-jits + re-uploads
        # consts per call, but correct anywhere)
        _CACHE.update(dict(nc=nc, consts=build_consts(), fallback=True))
        return _CACHE

    bass2jax.install_neuronx_cc_hook()

    partition_name = (nc.partition_id_tensor.name
                      if nc.partition_id_tensor else None)
    in_names = []
    out_names = []
    out_avals = []
    for alloc in nc.m.functions[0].allocations:
        if not isinstance(alloc, mybir.MemoryLocationSet):
            continue
        name = alloc.memorylocations[0].name
        if alloc.kind == "ExternalInput":
            if name != partition_name:
                in_names.append(name)
        elif alloc.kind == "ExternalOutput":
            out_names.append(name)
            out_avals.append(jax.core.ShapedArray(
                tuple(alloc.tensor_shape), mybir.dt.np(alloc.dtype)))
    all_in_names = list(in_names) + list(out_names)
    if partition_name is not None:
        all_in_names.append(partition_name)

    dbg_extra = {}
    if nc.dbg_addr is not None:
        dbg_extra[nc.dbg_addr.name] = np.zeros((1, 2), np.uint32)

    def _body(*args):
        operands = list(args)
        if partition_name is not None:
            operands.append(bass2jax.partition_id_tensor())
        outs = bass2jax._bass_exec_p.bind(
            *operands,
            out_avals=tuple(out_avals),
            in_names=tuple(all_in_names),
            out_names=tuple(out_names),
            lowering_input_output_aliases=(),
            sim_require_finite=True,
            sim_require_nnan=True,
            nc=nc,
        )
        return tuple(outs)

    devices = jax.devices()[:N_CORES]
    mesh = Mesh(np.asarray(devices), ("core",))
    n_args = len(in_names) + len(out_names)
    sharded = jax.jit(shard_map(
        _body, mesh=mesh,
        in_specs=(PartitionSpec("core"),) * n_args,
        out_specs=(PartitionSpec("core"),) * len(out_names),
        check_rep=False))
    shard = NamedSharding(mesh, PartitionSpec("core"))

    # device-resident weight-independent inputs: consts (replicated per
    # core) + zero output buffers (never donated, so they persist).
    consts = build_consts()
    static = {}
    for name in in_names:
        if name == "xw":
            continue
        if name in consts:
            a = consts[name]
        elif nc.dbg_addr is not None and name == nc.dbg_addr.name:
            a = dbg_extra[name]
        else:
            raise KeyError(f"unexpected input {name}")
        tiled = np.ascontiguousarray(
            np.broadcast_to(a, (N_CORES,) + a.shape).reshape(
                (N_CORES * a.shape[0],) + a.shape[1:]))
        static[name] = jax.device_put(tiled, shard)
    zero_bufs = {}
    for name, av in zip(out_names, out_avals):
        zero_bufs[name] = jax.device_put(
            np.zeros((N_CORES * av.shape[0],) + av.shape[1:], av.dtype),
            shard)
    for v in static.values():
        v.block_until_ready()
    for v in zero_bufs.values():
        v.block_until_ready()

    _CACHE.update(dict(nc=nc, sharded=sharded, shard=shard,
                       in_names=in_names, out_names=out_names,
                       static=static, zero_bufs=zero_bufs, jax=jax))
    return _CACHE


def _pack_payload(x, weight):
    """Pack u8-quantized x + weight + dequant scales into the payload."""
    x3 = x.reshape(N_CORES, B_LOC * CIN, L * L)
    xw = np.empty((N_CORES, PROWS, 1024), dtype=np.uint8)
    amax = np.maximum(np.maximum(x3.max(axis=2), -x3.min(axis=2)), 1e-20)
    tmp = _CACHE.get("pack_tmp")
    if tmp is None:
        tmp = _CACHE["pack_tmp"] = np.empty_like(x3)
    np.multiply(x3, (XQ / amax)[:, :, None], out=tmp)
    np.add(tmp, 128.5, out=tmp)
    # u8 cast truncates: floor(v + 128.5) == round(v) + 128, range [1, 255]
    np.copyto(xw[:, :XROWS, :], tmp, casting="unsafe")
    # wT[k1*9+k2, n*16+cin'] zero-padded to [128, 128] bf16, as raw bytes
    wt = np.zeros((128, 128), dtype=NPBF)
    wt[:81] = weight.transpose(2, 3, 0, 1).reshape(81, 128)
    xw[:, XROWS:XROWS + WROWS, :] = wt.view(np.uint8).reshape(32, 1024)
    # f32 dequant scales, laid out [c, b] per core
    sxc = np.ascontiguousarray(
        (amax / XQ).astype(np.float32).reshape(
            N_CORES, B_LOC, CIN).transpose(0, 2, 1))       # [8, 128, 4]
    xw[:, XROWS + WROWS:PROWS, :] = sxc.view(np.uint8).reshape(
        N_CORES, SROWS, 1024)
    return xw.reshape(N_CORES * PROWS, 1024)


def _dequant(raw):
    """u8 [B, 257, 1024] device result -> f32 [B, 256, 32, 32]."""
    inv = np.ascontiguousarray(raw[:, 256, :]).view(np.float32)  # [B, 256]
    scale = (1.0 / inv.astype(np.float64)).astype(np.float32)
    out = np.empty((B, 256, 1024), np.float32)
    np.subtract(raw[:, :256, :], np.float32(QOFF), out=out, casting="unsafe")
    np.multiply(out, scale[:, :, None], out=out)
    return out.reshape(B, 256, 32, 32)


def kernel(x, weight):
    x = np.asarray(x, dtype=np.float32)
    weight = np.asarray(weight, dtype=np.float32)
    if "sharded" not in _CACHE and "fallback" not in _CACHE:
        _setup()
    c = _CACHE

    if c.get("fallback"):
        from concourse.bass_utils import run_bass_kernel_spmd
        xw = _pack_payload(x, weight).reshape(N_CORES, PROWS, 1024)
        in_maps = []
        for i in range(N_CORES):
            m = {"xw": xw[i]}
            m.update(c["consts"])
            in_maps.append(m)
        res = run_bass_kernel_spmd(c["nc"], in_maps,
                                   core_ids=list(range(N_CORES)))
        raw = np.concatenate([r["out"] for r in res.results], axis=0)
        return _dequant(raw)

    # the np payload is passed straight to the jitted call (jax shards and
    # uploads it; an explicit device_put adds an extra RPC for no benefit)
    xw = _pack_payload(x, weight)
    args = []
    for name in c["in_names"]:
        args.append(xw if name == "xw" else c["static"][name])
    for name in c["out_names"]:
        args.append(c["zero_bufs"][name])
    outs = c["sharded"](*args)
    raw = np.asarray(outs[0])                       # u8 [B, 257, 1024]
    _CACHE["last_raw"] = raw
    return _dequant(raw)


if __name__ == "__main__":
    import jax

    sys.path.insert(0, "/root/problem")
    from reference import setup_inputs, reference

    with jax.default_device(jax.devices("cpu")[0]):
        inputs = setup_inputs()
        inputs = {k: np.asarray(v) for k, v in inputs.items()}
        expected = np.asarray(reference(**inputs))
    actual = kernel(**inputs)
    err = np.linalg.norm(actual - expected) / np.linalg.norm(expected)
    print("Relative error:", err)


# revision 34
# speedup vs baseline: 1.1637x; 1.1562x over previous
"""Trainium2 Bass kernel for FConv2d (FFT conv module), v5.

out = irfftn( rfftn(x, axes=(c,h,w)) * rfftn(pad(weight)) )[:, :, ::4] reshaped.

Device math (data-parallel over batch, 4 per core x 8 cores), bf16 data with
fp32 PSUM accumulation:
  W   on-device weight transform: spatial DFT of the 9x9 taps (contract 81)
      then channel DFT (contract 16), Gauss-packed into wpk for the MUL stage.
  T   on-device u8->bf16 shift + transpose of x[b] ([c, hw] -> [hw, c]
      chunks) via PE identity matmuls; host ships x in its native layout.
  S   joint 2D spatial rFFT while data is REAL: contract hw=1024 via 8
      PSUM-accumulated matmuls per batch; the per-(b, c) x-dequant scale is
      folded into the PSUM evacuation.
  C   channel DFT (contract c=128) producing X[k_c, f] for 544 spatial freqs.
  MUL complex multiply with W-hat via Gauss 3-mult on DVE (+Pool offload).
  I1  subsampled inverse channel DFT via paired matmuls.
  I2  joint 2D spatial inverse + Re extraction via paired matmuls, then
      per-channel abs-max u8 quantization of the output on device.

The wall-clock bottleneck is host<->device traffic: the NeuronCores sit
behind an axon tunnel moving ~35-50 MB/s with ~80 ms per-call dispatch
latency, while the device compute itself is ~0.2 ms. Per call this kernel
ships one 4.5 MB H2D payload (u8-quantized x + bf16 weight bytes + f32
scales) and one 8.9 MB D2H result (u8-quantized out + f32 per-channel
scales, dequantized on host). All DFT twiddle constants and the output
zero buffers are weight-independent, device-resident, and uploaded only
once; the jitted shard_map executable is cached across calls.
"""
import sys
import numpy as np

for _p in ("/opt/trn_rl_repo", "/root/.axon_site/_ro/trn_rl_repo"):
    if _p not in sys.path:
        sys.path.insert(0, _p)

import ml_dtypes

import concourse.bacc as bacc
import concourse.bass as bass
import concourse.mybir as mybir
import concourse.tile as tile
from concourse import bass2jax

F32 = mybir.dt.float32
BF16 = mybir.dt.bfloat16
U8 = mybir.dt.uint8
NPBF = ml_dtypes.bfloat16

# u8 output quantization: u = out * (QCLIP/absmax) + QOFF on device (DVE
# float->u8 convert), out = (u - QOFF) / inv on host. QCLIP < 127 leaves
# headroom so saturation/rounding at the extremes cannot wrap.
QCLIP = 126.5
QOFF = 128.0

B = 32          # full batch
B_LOC = 4       # per core
N_CORES = 8
CIN = 128
L = 32
NFIL = 8        # num filters n
NF = 544        # stored spatial freqs (32 * 17)
NFP = 640       # padded: 5 chunks of 128
XROWS = B_LOC * CIN          # 512 rows of u8-quantized x per core
WROWS = 32                   # 32 rows holding the packed [128,128] bf16 weight
SROWS = 2                    # 2 rows holding the 512 f32 dequant scales
PROWS = XROWS + WROWS + SROWS   # 546 payload rows per core

# x u8 quantization: q = round(x * 127/absmax) + 128 per (b, c) row on host;
# device subtracts 128 during the u8->bf16 convert and folds absmax/127 into
# the spatial-rFFT output (per-partition scale).
XQ = 127.0

# which n-pairs get their zr/zi add/sub on Pool instead of DVE
POOL_PAIRS = (0, 1, 2, 3)


# ----------------------------------------------------------------- constants
def build_consts():
    """Weight-independent DFT factor tensors (bf16). Uploaded once."""
    f = np.arange(NF)
    p = f // 17
    q = f % 17

    # S: joint real 2D rfft factor, per hw chunk t: [128 hw, 1088]
    # cols = [Re(f) 0:544 | Im(f) 544:1088];  Re = cos, Im = -sin
    f2 = np.zeros((128, 8, 1088), dtype=np.float64)
    hw_p = np.arange(128)
    for t in range(8):
        h = 4 * t + hw_p // 32
        w = hw_p % 32
        ang = 2 * np.pi * (np.outer(h, p) + np.outer(w, q)) / 32.0
        f2[:, t, 0:NF] = np.cos(ang)
        f2[:, t, NF:2 * NF] = -np.sin(ang)

    # C: channel DFT lhsT pack [c, 384] = [cos | sin | -sin]
    c = np.arange(128)
    k = np.arange(128)
    angc = 2 * np.pi * np.outer(c, k) / 128.0
    fc = np.concatenate(
        [np.cos(angc), np.sin(angc), -np.sin(angc)], axis=1)

    # I1 rhs: e1 = [cos|sin], e2 = [-sin|cos] at output positions c=4j
    j32 = np.arange(32)
    ange = 2 * np.pi * np.outer(k, j32) / 32.0
    er = np.cos(ange)
    ei = np.sin(ange)
    e1 = np.concatenate([er, ei], axis=1)
    e2 = np.concatenate([-ei, er], axis=1)

    # I2 rhs: k2d[128, 5, 2, 1024]: cos / -sin of inverse angles
    k2d = np.zeros((128, 5, 2, 1024), dtype=np.float64)
    yz = np.arange(1024)
    y = yz // 32
    z = yz % 32
    fp = np.arange(NFP)
    valid = (fp < NF).astype(np.float64)
    pv = np.zeros(NFP, dtype=np.int64)
    qv = np.zeros(NFP, dtype=np.int64)
    pv[:NF] = p
    qv[:NF] = q
    for fcb in range(5):
        sl = slice(fcb * 128, (fcb + 1) * 128)
        ang4 = 2 * np.pi * (np.outer(pv[sl], y) + np.outer(qv[sl], z)) / 32.0
        k2d[:, fcb, 0, :] = np.cos(ang4) * valid[sl][:, None]
        k2d[:, fcb, 1, :] = -np.sin(ang4) * valid[sl][:, None]

    # W stage step 1: spatial DFT of the 9x9 taps; rows = k1*9+k2 (pad to
    # 128), cols = [Re 544 | Im 544], with alpha[q]/(128*32*32) folded in.
    alpha = np.full(17, 2.0)
    alpha[0] = 1.0
    alpha[16] = 1.0
    scale = alpha[q] / (128.0 * 32.0 * 32.0)
    k12 = np.arange(81)
    k1 = k12 // 9
    k2 = k12 % 9
    angw = 2 * np.pi * (np.outer(k1, p) + np.outer(k2, q)) / 32.0
    fw = np.zeros((128, 1088), dtype=np.float64)
    fw[:81, 0:NF] = np.cos(angw) * scale
    fw[:81, NF:2 * NF] = -np.sin(angw) * scale

    # W stage step 2: channel DFT lhsT pack over cin'=16: [16, 384] =
    # [cos | sin | -sin] of 2*pi*c*cin'/128
    ci = np.arange(16)
    angec = 2 * np.pi * np.outer(ci, c) / 128.0
    ec = np.concatenate(
        [np.cos(angec), np.sin(angec), -np.sin(angec)], axis=1)

    # T stage: identity for PE transposes
    id128 = np.eye(128)

    cast = lambda a: np.ascontiguousarray(a).astype(NPBF)
    return {
        "f2": cast(f2),
        "fc": cast(fc),
        "e1": cast(e1),
        "e2": cast(e2),
        "k2d": cast(k2d),
        "fw": cast(fw),
        "ec": cast(ec),
        "id128": cast(id128),
    }


# ----------------------------------------------------------------- program
def build_program():
    nc = bacc.Bacc("TRN2", target_bir_lowering=False, debug=False)
    # xw rows 0:512 = u8-quantized x[b] as [c, h*w] (native layout); rows
    # 512:544 = weight packed [128, 128] bf16 (wT[k1*9+k2, n*16+cin'],
    # zero-padded) as raw bytes; rows 544:546 = the 512 f32 x-dequant scales
    # laid out [c, b].
    xw_d = nc.dram_tensor("xw", [PROWS, 1024], U8, kind="ExternalInput")
    f2_d = nc.dram_tensor("f2", [128, 8, 1088], BF16, kind="ExternalInput")
    fc_d = nc.dram_tensor("fc", [128, 384], BF16, kind="ExternalInput")
    e1_d = nc.dram_tensor("e1", [128, 64], BF16, kind="ExternalInput")
    e2_d = nc.dram_tensor("e2", [128, 64], BF16, kind="ExternalInput")
    k2d_d = nc.dram_tensor("k2d", [128, 5, 2, 1024], BF16, kind="ExternalInput")
    fw_d = nc.dram_tensor("fw", [128, 1088], BF16, kind="ExternalInput")
    ec_d = nc.dram_tensor("ec", [16, 384], BF16, kind="ExternalInput")
    id_d = nc.dram_tensor("id128", [128, 128], BF16, kind="ExternalInput")
    # rows 0:256 = u8-quantized out channels; row 256 = the 256 per-channel
    # f32 inverse scales (bitcast into the u8 row).
    out_d = nc.dram_tensor("out", [B_LOC, 257, 1024], U8,
                           kind="ExternalOutput")

    with tile.TileContext(nc) as tc:
        with (
            tc.tile_pool(name="consts", bufs=1) as cpool,
            tc.tile_pool(name="xin", bufs=2) as xpool,
            tc.tile_pool(name="xs", bufs=2) as xspool,
            tc.tile_pool(name="xc", bufs=2) as xcpool,
            tc.tile_pool(name="z", bufs=2) as zpool,
            tc.tile_pool(name="a", bufs=1) as apool,
            tc.tile_pool(name="o", bufs=3) as opool,
            tc.tile_pool(name="q", bufs=2) as qpool,
            tc.tile_pool(name="ps_f", bufs=1, space="PSUM") as pfpool,
            tc.tile_pool(name="ps_t", bufs=1, space="PSUM") as ptpool,
            tc.tile_pool(name="ps_i1", bufs=2, space="PSUM") as pi1pool,
            tc.tile_pool(name="ps_i2", bufs=2, space="PSUM") as pi2pool,
        ):
            # ---- constants, ordered by first use
            id_sb = cpool.tile([128, 128], BF16)
            nc.sync.dma_start(out=id_sb[:], in_=id_d[:])
            fw_sb = cpool.tile([128, 1088], BF16)
            nc.sync.dma_start(out=fw_sb[:], in_=fw_d[:])
            ec_sb = cpool.tile([16, 384], BF16)
            nc.sync.dma_start(out=ec_sb[:], in_=ec_d[:])
            f2_sb = cpool.tile([128, 8, 1088], BF16)
            for t in range(8):
                nc.sync.dma_start(out=f2_sb[:, t], in_=f2_d[:, t])
            fc_sb = cpool.tile([128, 384], BF16)
            nc.sync.dma_start(out=fc_sb[:], in_=fc_d[:])
            e1_sb = cpool.tile([128, 64], BF16)
            nc.sync.dma_start(out=e1_sb[:], in_=e1_d[:])
            e2_sb = cpool.tile([128, 64], BF16)
            nc.sync.dma_start(out=e2_sb[:], in_=e2_d[:])
            k2d_sb = cpool.tile([128, 5, 2, 1024], BF16)
            for fcb in range(5):
                nc.sync.dma_start(out=k2d_sb[:, fcb], in_=k2d_d[:, fcb])
            wpk_sb = cpool.tile([128, 4, 3, 2, NF], BF16)
            wh_sb = cpool.tile([128, 2, NF], BF16)

            # ---- W: on-device weight transform -> wpk_sb
            # wpk[128 c, pair, kind, n_in_pair, 544]; kinds: A=wr, B=wr+wi,
            # C=wi-wr (alpha/N normalization folded into fw).
            wt_sb = cpool.tile([128, 128], BF16)
            nc.sync.dma_start(
                out=wt_sb[:],
                in_=xw_d[XROWS:XROWS + WROWS].rearrange(
                    "a (b c) -> (a b) c", b=4, c=256).bitcast(BF16))
            sx_sb = cpool.tile([128, 4], F32)
            nc.sync.dma_start(
                out=sx_sb[:],
                in_=xw_d[XROWS + WROWS:PROWS].rearrange(
                    "a (b c) -> (a b) c", b=64, c=16).bitcast(F32))
            # step 1: spatial DFT (contract k12=81, zero-padded to 128),
            # one [16 cin', 1088] output per filter n so step 2's rhs sits
            # at SBUF partition base 0.
            ws_sb = cpool.tile([16, NFIL, 1088], BF16)
            for n in range(NFIL):
                ws0 = pfpool.tile([128, 512], F32, tag="f0", name=f"ws0_{n}")
                ws1 = pfpool.tile([128, 512], F32, tag="f1", name=f"ws1_{n}")
                ws2 = pfpool.tile([128, 64], F32, tag="f2", name=f"ws2_{n}")
                lw = wt_sb[:, n * 16:(n + 1) * 16]
                nc.tensor.matmul(ws0[0:16, :], lw, fw_sb[:, 0:512],
                                 start=True, stop=True)
                nc.tensor.matmul(ws1[0:16, :], lw, fw_sb[:, 512:1024],
                                 start=True, stop=True)
                nc.tensor.matmul(ws2[0:16, :], lw, fw_sb[:, 1024:1088],
                                 start=True, stop=True)
                nc.scalar.copy(ws_sb[:, n, 0:512], ws0[0:16, :])
                nc.scalar.copy(ws_sb[:, n, 512:1024], ws1[0:16, :])
                nc.scalar.copy(ws_sb[:, n, 1024:1088], ws2[0:16, :])
            # step 2 per n: channel DFT (contract cin'=16) + Gauss pack.
            # Wr = cosT@Wsr + sinT@Wsi ; Wi = cosT@Wsi - sinT@Wsr
            lcos = ec_sb[:, 0:128]
            lsin = ec_sb[:, 128:256]
            lnsin = ec_sb[:, 256:384]
            for n in range(NFIL):
                pr, o = n // 2, n % 2
                re0 = ws_sb[:, n, 0:512]
                re1 = ws_sb[:, n, 512:544]
                im0 = ws_sb[:, n, 544:1056]
                im1 = ws_sb[:, n, 1056:1088]
                wr0 = pfpool.tile([128, 512], F32, tag="f0", name=f"wr0_{n}")
                wi0 = pfpool.tile([128, 512], F32, tag="f1", name=f"wi0_{n}")
                wx1 = pfpool.tile([128, 64], F32, tag="f2", name=f"wx1_{n}")
                nc.tensor.matmul(wr0[:], lcos, re0, start=True, stop=False)
                nc.tensor.matmul(wr0[:], lsin, im0, start=False, stop=True)
                nc.tensor.matmul(wx1[:, 0:32], lcos, re1,
                                 start=True, stop=False, skip_group_check=True)
                nc.tensor.matmul(wx1[:, 0:32], lsin, im1,
                                 start=False, stop=True, skip_group_check=True)
                nc.tensor.matmul(wi0[:], lcos, im0, start=True, stop=False)
                nc.tensor.matmul(wi0[:], lnsin, re0, start=False, stop=True)
                nc.tensor.matmul(wx1[:, 32:64], lcos, im1,
                                 start=True, stop=False, skip_group_check=True)
                nc.tensor.matmul(wx1[:, 32:64], lnsin, re1,
                                 start=False, stop=True, skip_group_check=True)
                # pack: A=wr, B=wr+wi, C=wi-wr (bf16). DVE can't read two
                # PSUM operands, so stage wr/wi through SBUF first.
                nc.scalar.copy(wh_sb[:, 0, 0:512], wr0[:])
                nc.scalar.copy(wh_sb[:, 0, 512:544], wx1[:, 0:32])
                nc.scalar.copy(wh_sb[:, 1, 0:512], wi0[:])
                nc.scalar.copy(wh_sb[:, 1, 512:544], wx1[:, 32:64])
                nc.gpsimd.tensor_scalar_mul(wpk_sb[:, pr, 0, o],
                                            wh_sb[:, 0], 1.0)
                nc.vector.tensor_add(wpk_sb[:, pr, 1, o],
                                     wh_sb[:, 0], wh_sb[:, 1])
                nc.vector.tensor_sub(wpk_sb[:, pr, 2, o],
                                     wh_sb[:, 1], wh_sb[:, 0])

            def emit_forward(b):
                # ---- load u8 x[b] in native [c, hw] layout, dequant-shift
                # to bf16 ints, transpose on PE
                xb8 = xpool.tile([128, 1024], U8, tag="xb8")
                nc.gpsimd.dma_start(
                    out=xb8[:], in_=xw_d[b * 128:(b + 1) * 128, :])
                xb = xpool.tile([128, 1024], BF16, tag="xb")
                nc.vector.tensor_scalar_add(xb[:], xb8[:], -128.0)
                xt = xpool.tile([128, 8, 128], BF16, tag="xt")
                for t in range(8):
                    pt = ptpool.tile([128, 128], BF16, tag="pt")
                    nc.tensor.transpose(
                        pt[:], xb[:, t * 128:(t + 1) * 128], id_sb[:])
                    nc.scalar.copy(xt[:, t, :], pt[:])

                # ---- S: joint real 2D rfft: Xs[c, 1088] in 3 PSUM tiles
                sp0 = pfpool.tile([128, 512], F32, tag="f0", name=f"sp0_{b}")
                sp1 = pfpool.tile([128, 512], F32, tag="f1", name=f"sp1_{b}")
                sp2 = pfpool.tile([128, 64], F32, tag="f2", name=f"sp2_{b}")
                for t in range(8):
                    st = (t == 0)
                    sp = (t == 7)
                    lhsT = xt[:, t, :]
                    nc.tensor.matmul(sp0[:], lhsT, f2_sb[:, t, 0:512],
                                     start=st, stop=sp)
                    nc.tensor.matmul(sp1[:], lhsT, f2_sb[:, t, 512:1024],
                                     start=st, stop=sp)
                    nc.tensor.matmul(sp2[:], lhsT, f2_sb[:, t, 1024:1088],
                                     start=st, stop=sp)
                # fold the per-(b, c) x-dequant scale in while evacuating
                # PSUM (Xs partitions = input channel c)
                sxb = sx_sb[:, b:b + 1]
                xs_sb = xspool.tile([128, 1088], BF16, tag="xs")
                nc.vector.tensor_scalar_mul(xs_sb[:, 0:512], sp0[:], sxb)
                nc.vector.tensor_scalar_mul(xs_sb[:, 512:1024], sp1[:], sxb)
                nc.vector.tensor_scalar_mul(xs_sb[:, 1024:1088], sp2[:], sxb)

                # ---- C: channel DFT -> Xr/Xi [k_c, 544]
                # Xr = cosT@Xsr + sinT@Xsi ; Xi = cosT@Xsi - sinT@Xsr
                cp0 = pfpool.tile([128, 512], F32, tag="f0", name=f"cp0_{b}")
                cp1 = pfpool.tile([128, 512], F32, tag="f1", name=f"cp1_{b}")
                cp2 = pfpool.tile([128, 64], F32, tag="f2", name=f"cp2_{b}")
                ccos = fc_sb[:, 0:128]
                csin = fc_sb[:, 128:256]
                cnsin = fc_sb[:, 256:384]
                nc.tensor.matmul(cp0[:], ccos, xs_sb[:, 0:512],
                                 start=True, stop=False)
                nc.tensor.matmul(cp0[:], csin, xs_sb[:, 544:1056],
                                 start=False, stop=True)
                nc.tensor.matmul(cp2[:, 0:32], ccos, xs_sb[:, 512:544],
                                 start=True, stop=False, skip_group_check=True)
                nc.tensor.matmul(cp2[:, 0:32], csin, xs_sb[:, 1056:1088],
                                 start=False, stop=True, skip_group_check=True)
                nc.tensor.matmul(cp1[:], ccos, xs_sb[:, 544:1056],
                                 start=True, stop=False)
                nc.tensor.matmul(cp1[:], cnsin, xs_sb[:, 0:512],
                                 start=False, stop=True)
                nc.tensor.matmul(cp2[:, 32:64], ccos, xs_sb[:, 1056:1088],
                                 start=True, stop=False, skip_group_check=True)
                nc.tensor.matmul(cp2[:, 32:64], cnsin, xs_sb[:, 512:544],
                                 start=False, stop=True, skip_group_check=True)

                xr = xcpool.tile([128, NF], BF16, tag="xr")
                xi = xcpool.tile([128, NF], BF16, tag="xi")
                xsum = xcpool.tile([128, NF], BF16, tag="xsum")
                nc.vector.tensor_scalar_mul(xr[:, 0:512], cp0[:], 1.0)
                nc.vector.tensor_scalar_mul(xr[:, 512:544], cp2[:, 0:32], 1.0)
                nc.vector.tensor_scalar_mul(xi[:, 0:512], cp1[:], 1.0)
                nc.vector.tensor_scalar_mul(xi[:, 512:544], cp2[:, 32:64], 1.0)
                nc.vector.tensor_add(xsum[:], xr[:], xi[:])

                # ---- MUL per n-pair (z for this b, consumed by back(b))
                zs = []
                for pr in range(4):
                    zr = zpool.tile([128, 2, NFP], BF16, tag=f"zr{pr}", bufs=3)
                    zi = zpool.tile([128, 2, NFP], BF16, tag=f"zi{pr}", bufs=3)
                    k1 = zpool.tile([128, 2, NF], BF16, tag=f"k1{pr}", bufs=2)
                    nc.gpsimd.memset(zr[:, :, NF:NFP].bitcast(F32), 0.0)
                    nc.gpsimd.memset(zi[:, :, NF:NFP].bitcast(F32), 0.0)
                    bc = lambda ap: ap.rearrange(
                        "p (o f) -> p o f", o=1).broadcast_to([128, 2, NF])
                    zrv = zr[:, :, 0:NF]
                    ziv = zi[:, :, 0:NF]
                    # zr = k1 - (wr+wi)*xi ; zi = k1 + (wi-wr)*xr
                    nc.vector.tensor_mul(ziv, bc(xr[:]), wpk_sb[:, pr, 2])
                    nc.vector.tensor_mul(zrv, bc(xi[:]), wpk_sb[:, pr, 1])
                    nc.vector.tensor_mul(k1[:], bc(xsum[:]), wpk_sb[:, pr, 0])
                    eng = nc.gpsimd if pr in POOL_PAIRS else nc.vector
                    eng.tensor_sub(zrv, k1[:], zrv)
                    eng.tensor_add(ziv, k1[:], ziv)
                    zs.append((zr, zi))
                return zs

            def emit_back(b, zs, last=False):
                # ---- I1 per n
                a_sb = apool.tile([128, 5, 2, NFIL, 32], BF16, tag="a")
                for pr in range(4):
                    zr, zi = zs[pr]
                    for o in range(2):
                        n = 2 * pr + o
                        ips = pi1pool.tile([128, 320], F32, tag="ips")
                        for fcb in range(5):
                            col = slice(fcb * 64, (fcb + 1) * 64)
                            zsl = slice(fcb * 128, (fcb + 1) * 128)
                            nc.tensor.matmul(
                                ips[:, col], zr[:, o, zsl], e1_sb[:],
                                start=True, stop=False)
                            nc.tensor.matmul(
                                ips[:, col], zi[:, o, zsl], e2_sb[:],
                                start=False, stop=True)
                        nc.scalar.copy(
                            a_sb[:, :, :, n, :],
                            ips[:].rearrange("p (fc c j) -> p fc c j",
                                             fc=5, c=2))

                # ---- I2: joint 2D inverse + Re extraction + u8 quantization
                for mh in range(2):
                    opsl = []
                    for nzc in range(2):
                        ops = pi2pool.tile([128, 512], F32, tag="psi2")
                        for fcb in range(5):
                            for comp in range(2):
                                lhsT = a_sb[:, fcb, comp,
                                            mh * 4:(mh + 1) * 4, :]
                                rhs = k2d_sb[:, fcb, comp,
                                             nzc * 512:(nzc + 1) * 512]
                                nc.tensor.matmul(
                                    ops[:], lhsT, rhs,
                                    start=(fcb == 0 and comp == 0),
                                    stop=(fcb == 4 and comp == 1))
                        opsl.append(ops)
                    # per-channel (partition) abs-max over the 1024 spatial
                    # values, then inv = QCLIP / absmax
                    am = qpool.tile([128, 4], F32, tag="am")
                    nc.vector.tensor_reduce(
                        am[:, 0:1], opsl[0][:], axis=mybir.AxisListType.X,
                        op=mybir.AluOpType.max, apply_absolute_value=True)
                    nc.vector.tensor_reduce(
                        am[:, 1:2], opsl[1][:], axis=mybir.AxisListType.X,
                        op=mybir.AluOpType.max, apply_absolute_value=True)
                    nc.vector.tensor_max(am[:, 2:3], am[:, 0:1], am[:, 1:2])
                    nc.vector.tensor_scalar_max(am[:, 2:3], am[:, 2:3], 1e-20)
                    inv = qpool.tile([128, 1], F32, tag="inv")
                    nc.vector.reciprocal(inv[:], am[:, 2:3])
                    nc.vector.tensor_scalar_mul(inv[:], inv[:], QCLIP)
                    # quantize both 512-chunks: u8 = ops * inv + QOFF
                    o_sb = opool.tile([128, 1024], U8, tag="o")
                    nc.vector.tensor_scalar(
                        o_sb[:, 0:512], opsl[0][:], inv[:], QOFF,
                        op0=mybir.AluOpType.mult, op1=mybir.AluOpType.add)
                    nc.vector.tensor_scalar(
                        o_sb[:, 512:1024], opsl[1][:], inv[:], QOFF,
                        op0=mybir.AluOpType.mult, op1=mybir.AluOpType.add)
                    nc.sync.dma_start(
                        out=out_d[b, mh * 128:(mh + 1) * 128, :],
                        in_=o_sb[:])
                    # ship the f32 inv factors in row 256
                    inv_dst = out_d[b, 256].bitcast(F32).rearrange(
                        "(p one) -> p one", one=1)[mh * 128:(mh + 1) * 128]
                    nc.sync.dma_start(out=inv_dst, in_=inv[:])

            # ---- software-pipelined schedule, depth 1:
            # fwd(b) runs one PE window ahead of back(b)
            zs_all = {}
            DEPTH = 1
            for b in range(B_LOC):
                zs_all[b] = emit_forward(b)
                if b >= DEPTH:
                    emit_back(b - DEPTH, zs_all[b - DEPTH])
            for b in range(B_LOC - DEPTH, B_LOC):
                emit_back(b, zs_all[b], last=(b == B_LOC - 1))
    nc.compile()
    return nc


# ----------------------------------------------------------------- host side
_CACHE = {}


def _setup():
    """Build program + cached jit + device-resident consts. Runs once."""
    import jax
    import jax.numpy as jnp  # noqa: F401
    from jax.sharding import Mesh, PartitionSpec, NamedSharding
    from jax.experimental.shard_map import shard_map
    from concourse._compat import axon_active

    try:
        # persistent executable cache: makes the first call in a fresh
        # process load the compiled NEFF-wrapped executable from disk
        # instead of recompiling (~1-3 min). Harmless no-op on miss.
        jax.config.update("jax_compilation_cache_dir",
                          "/root/.cache/jax_bass_cc")
        jax.config.update("jax_persistent_cache_min_compile_time_secs", 2.0)
    except Exception:
        pass

    nc = build_program()
    if not axon_active():
        # native /dev/neuron* path: no bass_exec custom-call executor, so
        # run through run_bass_kernel_spmd (slower: re-jits + re-uploads
        # consts per call, but correct anywhere)
        _CACHE.update(dict(nc=nc, consts=build_consts(), fallback=True))
        return _CACHE

    bass2jax.install_neuronx_cc_hook()

    partition_name = (nc.partition_id_tensor.name
                      if nc.partition_id_tensor else None)
    in_names = []
    out_names = []
    out_avals = []
    for alloc in nc.m.functions[0].allocations:
        if not isinstance(alloc, mybir.MemoryLocationSet):
            continue
        name = alloc.memorylocations[0].name
        if alloc.kind == "ExternalInput":
            if name != partition_name:
                in_names.append(name)
        elif alloc.kind == "ExternalOutput":
            out_names.append(name)
            out_avals.append(jax.core.ShapedArray(
                tuple(alloc.tensor_shape), mybir.dt.np(alloc.dtype)))
    all_in_names = list(in_names) + list(out_names)
    if partition_name is not None:
        all_in_names.append(partition_name)

    dbg_extra = {}
    if nc.dbg_addr is not None:
        dbg_extra[nc.dbg_addr.name] = np.zeros((1, 2), np.uint32)

    def _body(*args):
        operands = list(args)
        if partition_name is not None:
            operands.append(bass2jax.partition_id_tensor())
        outs = bass2jax._bass_exec_p.bind(
            *operands,
            out_avals=tuple(out_avals),
            in_names=tuple(all_in_names),
            out_names=tuple(out_names),
            lowering_input_output_aliases=(),
            sim_require_finite=True,
            sim_require_nnan=True,
            nc=nc,
        )
        return tuple(outs)

    devices = jax.devices()[:N_CORES]
    mesh = Mesh(np.asarray(devices), ("core",))
    n_args = len(in_names) + len(out_names)
    sharded = jax.jit(shard_map(
        _body, mesh=mesh,
        in_specs=(PartitionSpec("core"),) * n_args,
        out_specs=(PartitionSpec("core"),) * len(out_names),
        check_rep=False))
    shard = NamedSharding(mesh, PartitionSpec("core"))

    # device-resident weight-independent inputs: consts (replicated per
    # core) + zero output buffers (never donated, so they persist).
    consts = build_consts()
    static = {}
    for name in in_names:
        if name == "xw":
            continue
        if name in consts:
            a = consts[name]
        elif nc.dbg_addr is not None and name == nc.dbg_addr.name:
            a = dbg_extra[name]
        else:
            raise KeyError(f"unexpected input {name}")
        tiled = np.ascontiguousarray(
            np.broadcast_to(a, (N_CORES,) + a.shape).reshape(
                (N_CORES * a.shape[0],) + a.shape[1:]))
        static[name] = jax.device_put(tiled, shard)
    zero_bufs = {}
    for name, av in zip(out_names, out_avals):
        zero_bufs[name] = jax.device_put(
            np.zeros((N_CORES * av.shape[0],) + av.shape[1:], av.dtype),
            shard)
    for v in static.values():
        v.block_until_ready()
    for v in zero_bufs.values():
        v.block_until_ready()

    _CACHE.update(dict(nc=nc, sharded=sharded, shard=shard,
                       in_names=in_names, out_names=out_names,
                       static=static, zero_bufs=zero_bufs, jax=jax))
    return _CACHE


def _pack_payload(x, weight):
    """Pack u8-quantized x + weight + dequant scales into the payload."""
    x3 = x.reshape(N_CORES, B_LOC * CIN, L * L)
    xw = np.empty((N_CORES, PROWS, 1024), dtype=np.uint8)
    amax = np.maximum(np.maximum(x3.max(axis=2), -x3.min(axis=2)), 1e-20)
    tmp = _CACHE.get("pack_tmp")
    if tmp is None:
        tmp = _CACHE["pack_tmp"] = np.empty_like(x3)
    np.multiply(x3, (XQ / amax)[:, :, None], out=tmp)
    np.add(tmp, 128.5, out=tmp)
    # u8 cast truncates: floor(v + 128.5) == round(v) + 128, range [1, 255]
    np.copyto(xw[:, :XROWS, :], tmp, casting="unsafe")
    # wT[k1*9+k2, n*16+cin'] zero-padded to [128, 128] bf16, as raw bytes
    wt = np.zeros((128, 128), dtype=NPBF)
    wt[:81] = weight.transpose(2, 3, 0, 1).reshape(81, 128)
    xw[:, XROWS:XROWS + WROWS, :] = wt.view(np.uint8).reshape(32, 1024)
    # f32 dequant scales, laid out [c, b] per core
    sxc = np.ascontiguousarray(
        (amax / XQ).astype(np.float32).reshape(
            N_CORES, B_LOC, CIN).transpose(0, 2, 1))       # [8, 128, 4]
    xw[:, XROWS + WROWS:PROWS, :] = sxc.view(np.uint8).reshape(
        N_CORES, SROWS, 1024)
    return xw.reshape(N_CORES * PROWS, 1024)


def _dequant_into(raw, out):
    """u8 [n, 257, 1024] device rows -> f32 [n, 256, 1024] slab of out."""
    inv = np.ascontiguousarray(raw[:, 256, :]).view(np.float32)  # [n, 256]
    scale = (1.0 / inv.astype(np.float64)).astype(np.float32)
    np.subtract(raw[:, :256, :], np.float32(QOFF), out=out, casting="unsafe")
    np.multiply(out, scale[:, :, None], out=out)


def _dequant(raw):
    """u8 [B, 257, 1024] device result -> f32 [B, 256, 32, 32]."""
    out = np.empty((B, 256, 1024), np.float32)
    _dequant_into(raw, out)
    return out.reshape(B, 256, 32, 32)


def _fetch_dequant(arr):
    """Fetch each output shard as it lands and dequant it while later
    shards are still streaming down the tunnel (IO-bound threads)."""
    p = _CACHE.get("pool")
    if p is None:
        import concurrent.futures
        p = _CACHE["pool"] = concurrent.futures.ThreadPoolExecutor(
            max_workers=N_CORES)
    out = np.empty((B, 256, 1024), np.float32)

    def one(s):
        raw = np.asarray(s.data)            # blocks until this shard arrives
        b0 = s.index[0].start or 0
        _dequant_into(raw, out[b0:b0 + raw.shape[0]])

    list(p.map(one, arr.addressable_shards))
    return out.reshape(B, 256, 32, 32)


def kernel(x, weight):
    x = np.asarray(x, dtype=np.float32)
    weight = np.asarray(weight, dtype=np.float32)
    if "sharded" not in _CACHE and "fallback" not in _CACHE:
        _setup()
    c = _CACHE

    if c.get("fallback"):
        from concourse.bass_utils import run_bass_kernel_spmd
        xw = _pack_payload(x, weight).reshape(N_CORES, PROWS, 1024)
        in_maps = []
        for i in range(N_CORES):
            m = {"xw": xw[i]}
            m.update(c["consts"])
            in_maps.append(m)
        res = run_bass_kernel_spmd(c["nc"], in_maps,
                                   core_ids=list(range(N_CORES)))
        raw = np.concatenate([r["out"] for r in res.results], axis=0)
        return _dequant(raw)

    # the np payload is passed straight to the jitted call (jax shards and
    # uploads it; an explicit device_put adds an extra RPC for no benefit)
    xw = _pack_payload(x, weight)
    args = []
    for name in c["in_names"]:
        args.append(xw if name == "xw" else c["static"][name])
    for name in c["out_names"]:
        args.append(c["zero_bufs"][name])
    outs = c["sharded"](*args)
    return _fetch_dequant(outs[0])


if __name__ == "__main__":
    import jax

    sys.path.insert(0, "/root/problem")
    from reference import setup_inputs, reference

    with jax.default_device(jax.devices("cpu")[0]):
        inputs = setup_inputs()
        inputs = {k: np.asarray(v) for k, v in inputs.items()}
        expected = np.asarray(reference(**inputs))
    actual = kernel(**inputs)
    err = np.linalg.norm(actual - expected) / np.linalg.norm(expected)
    print("Relative error:", err)
